# revision 15
# baseline (speedup 1.0000x reference)
"""Trainium2 Bass kernel for nn_BiDGNBlock (moe_routing).

Data-parallel over batch across 8 NeuronCores (no collectives). Each core
computes one batch element end-to-end.

Structure (v2):
  - Expert table We streamed as fp8e4m3 (x64 prescaled; final LN is
    scale-invariant) -> 4.2MB. Expert matmuls in DoubleRow fp8 perf mode.
    Activations split hi+lo fp8e4m3 (two DR passes) so only the weight
    quantization error remains (~3.6% RMS -> ~1.5e-2 final rel).
  - Attention computes transposed outputs directly (out.T = v.T-style
    matmuls against scaled expE), with the softmax denominator and the
    bias-fold factors applied as per-row scalings folded into the
    projection stage. Both sides stacked as 128 rows everywhere.
  - Router: top-2 membership is invariant to the xp row norms, so the
    whole normalization chain is skipped; masks come from
    sim >= second_max per channel (is_ge), transposed once on the PE.
  - Mask bytes (0x00/0xFF) replicated across partitions via a small DRAM
    round-trip on the scalar engine's DMA queue; applied to fp8 acts with
    uint32 bitwise-AND ops on the DVE.
  - Replicated LN vectors loaded with partition-broadcast DMA reads on the
    scalar queue (no PE/DVE cost, no host-side replication bytes).
Routing stays exact-fp32 end-to-end.
"""

import sys
import numpy as np

sys.path.insert(0, "/opt/trn_rl_repo")

N_CORES = 8
B, C, T = 8, 64, 256
EXP = 32
KT = T // 128
WE_SCALE = 64.0

_CACHE: dict = {}

BLOB_A_SPEC = [
    ("xtl", 128, (128, KT, C)), ("mt", 128, (128, KT, T)),
    ("xtr", 128, (128, KT, C)), ("wvt", 128, (128, KT, T)),
    ("xlr", 128, (128, T)), ("ident", 128, (128, 128)),
    ("w1t", 128, (128, KT, 1)),
]
BLOB_B_SPEC = [
    ("wpt", 128, (128, KT, T)), ("wrt", 128, (128, 2 * KT, EXP)),
    ("brp", 32, (32, 1)), ("cent", 32, (32, C)),
]
VEC_ROWS = ["bv", "bp", "agl", "agr", "abl", "abr", "mgl", "mgr",
            "mbl", "mbr"]


def _blob_layout():
    off = {}
    na = 0
    for name, parts, shape in BLOB_A_SPEC:
        cols = int(np.prod(shape[1:]))
        off[name] = (na, parts, shape)
        na += cols
    nb = 0
    for name, parts, shape in BLOB_B_SPEC:
        cols = int(np.prod(shape[1:]))
        off[name] = (nb, parts, shape)
        nb += cols
    return off, na, nb


BLOB_OFF, NA_COLS, NB_COLS = _blob_layout()


def _build():
    import concourse.bass as bass
    import concourse.mybir as mybir
    import concourse.tile as tile
    from concourse import bacc
    from contextlib import ExitStack

    dt = mybir.dt
    f32, f16, f8, u8 = dt.float32, dt.float16, dt.float8e4, dt.uint8
    u32 = dt.uint32
    AF = mybir.ActivationFunctionType
    OP = mybir.AluOpType
    DR = mybir.MatmulPerfMode.DoubleRow

    nc = bacc.Bacc("TRN2", target_bir_lowering=False, debug=False,
                   num_devices=N_CORES)

    def inp(name, shape, d=f32):
        return nc.dram_tensor(name, list(shape), d, kind="ExternalInput")

    blobA_d = inp("blobA", (128, NA_COLS))
    blobB_d = inp("blobB", (128, NB_COLS))
    vecs_d = inp("vecsd", (1, 10 * T))
    weh_d = inp("weh", (128, C, KT, T), f8)
    beh_d = inp("beh", (C, T), f16)

    olr_d = nc.dram_tensor("olr", [128, T], f32, kind="ExternalOutput")

    with tile.TileContext(nc) as tc, ExitStack() as ctx:
        cst = ctx.enter_context(tc.tile_pool(name="cst", bufs=1))
        wk = ctx.enter_context(tc.tile_pool(name="wk", bufs=2))
        sm = ctx.enter_context(tc.tile_pool(name="sm", bufs=2))
        asc_p = ctx.enter_context(tc.tile_pool(name="asc", bufs=3))
        ps = ctx.enter_context(tc.tile_pool(name="ps", bufs=3, space="PSUM"))
        ps_moe_p = ctx.enter_context(tc.tile_pool(name="psmoe", bufs=1,
                                                  space="PSUM"))
        dram = ctx.enter_context(tc.tile_pool(name="dram", bufs=1,
                                              space="DRAM"))

        # ---- input DMAs: blobA (attention-critical) first on sync queue ----
        blobA = cst.tile([128, NA_COLS], f32, tag="blobA")
        nc.sync.dma_start(out=blobA, in_=blobA_d.ap())
        blobB = cst.tile([128, NB_COLS], f32, tag="blobB")
        nc.sync.dma_start(out=blobB, in_=blobB_d.ap())
        we_sb = cst.tile([128, C, KT, T], f8, tag="weh")
        wea = weh_d.ap()
        for ch in range(8):
            nc.sync.dma_start(out=we_sb[:, ch * 8:(ch + 1) * 8],
                              in_=wea[:, ch * 8:(ch + 1) * 8])
        beh = cst.tile([C, T], f16, tag="beh")
        nc.sync.dma_start(out=beh, in_=beh_d.ap())

        # replicated LN vectors via partition-broadcast reads (scalar queue)
        vsrc = vecs_d.ap()

        def vec_bcast2(i_l, i_r, tag):
            t_sb = cst.tile([128, T], f32, tag=tag)
            src = bass.AP(tensor=vsrc.tensor, offset=vsrc.offset + i_l * T,
                          ap=[[T, 2], [0, 64], [1, T]])
            nc.scalar.dma_start(out=t_sb, in_=src)
            return t_sb

        bp_t = vec_bcast2(1, 1, "bp")
        g_lr = vec_bcast2(2, 3, "glr")
        b_lr = vec_bcast2(4, 5, "blr")
        mg_lr = vec_bcast2(6, 7, "mglr")
        mb_lr = vec_bcast2(8, 9, "mblr")
        bv_t = cst.tile([64, T], f32, tag="bv")
        nc.scalar.dma_start(
            out=bv_t, in_=bass.AP(tensor=vsrc.tensor, offset=vsrc.offset,
                                  ap=[[0, 64], [1, T]]))

        def bview(blob, name):
            off, parts, shape = BLOB_OFF[name]
            cols = 1
            for s in shape[1:]:
                cols *= s
            v = blob[0:parts, off:off + cols]
            if len(shape) == 3:
                v = v.rearrange("p (a b) -> p a b", a=shape[1])
            return v

        xtl = bview(blobA, "xtl")
        xtr = bview(blobA, "xtr")
        mt = bview(blobA, "mt")
        wvt = bview(blobA, "wvt")
        xlr = bview(blobA, "xlr")
        ident = bview(blobA, "ident")
        w1t = bview(blobA, "w1t")
        wpt = bview(blobB, "wpt")
        wrt = bview(blobB, "wrt")
        brp = bview(blobB, "brp")
        cent = bview(blobB, "cent")

        eps_t = cst.tile([128, 1], f32, tag="eps")
        nc.vector.memset(eps_t, 1e-5)
        onescol = cst.tile([64, 1], f32, tag="onescol")
        nc.vector.memset(onescol, 1.0)

        # PE warm-up + ACT table preloads during the DMA window
        warm_p = ctx.enter_context(tc.tile_pool(name="warm", bufs=1,
                                                space="PSUM"))
        wsrc = cst.tile([128, 512], f16, tag="wsrc")
        nc.vector.memset(wsrc, 0.5)
        pw = warm_p.tile([128, 512], f32, tag="warm")
        for wi in range(6):
            nc.tensor.matmul(pw, wsrc[:, 0:128], wsrc,
                             start=True, stop=True, skip_group_check=True)
        wact = cst.tile([1, 32], f32, tag="wact")
        nc.vector.memset(wact, 1.0)
        nc.scalar.activation(out=wact, in_=wact, func=AF.Exp)
        nc.scalar.activation(out=wact, in_=wact, func=AF.Sqrt)

        # ---- attention ----
        # A.T = (x_l @ M).T  [u, kt, cq]
        AT = wk.tile([128, KT, C], f32, tag="AT")
        for uo in range(KT):
            p = ps.tile([128, C], f32, tag="ps")
            for kt in range(KT):
                nc.tensor.matmul(p, mt[:, kt, uo * 128:(uo + 1) * 128],
                                 xtl[:, kt], start=(kt == 0),
                                 stop=(kt == KT - 1))
            nc.vector.tensor_copy(AT[:, uo], p)
        # energy E = A @ x_r.T
        pe_ = ps.tile([C, C], f32, tag="ps")
        for kt in range(KT):
            nc.tensor.matmul(pe_, AT[:, kt], xtr[:, kt],
                             start=(kt == 0), stop=(kt == KT - 1))
        # expE = exp(E/16)  (|E|/16 small enough to skip max-subtract)
        expE = wk.tile([C, C], f32, tag="expE")
        nc.scalar.activation(out=expE, in_=pe_, func=AF.Exp, scale=1.0 / 16.0)
        # rv.T = x_r @ (Wk.T bq); grv = exp(rv/16)  (bias fold, g-column)
        prv = ps.tile([C, 1], f32, tag="ps")
        for kt in range(KT):
            nc.tensor.matmul(prv, xtr[:, kt], w1t[:, kt],
                             start=(kt == 0), stop=(kt == KT - 1))
        grv = sm.tile([C, 1], f32, tag="grv")
        nc.scalar.activation(out=grv, in_=prv, func=AF.Exp, scale=1.0 / 16.0)
        # v = (x_l - x_r) @ Wv.T + bv   [ck, u]
        xdt = wk.tile([128, KT, C], f32, tag="xdt")
        nc.vector.tensor_sub(xdt, xtl, xtr)
        pv = ps.tile([C, T], f32, tag="ps")
        for kt in range(KT):
            nc.tensor.matmul(pv, xdt[:, kt], wvt[:, kt],
                             start=(kt == 0), stop=(kt == KT - 1))
        v_sb = wk.tile([C, T], f32, tag="v")
        nc.vector.tensor_tensor(out=v_sb, in0=pv, in1=bv_t, op=OP.add)

        # expEg_T[ck, cq] = expE[cq, ck] * g[ck]  (transpose + g-fold)
        pet = ps.tile([C, C], f32, tag="ps")
        nc.tensor.transpose(pet, expE, ident[0:C, 0:C])
        expEgT = wk.tile([C, C], f32, tag="expEgT")
        nc.vector.tensor_scalar(out=expEgT, in0=pet, scalar1=grv,
                                scalar2=None, op0=OP.mult)
        # S[cq] = col sums of expEg_T;  recipL = 1/S
        pS = ps.tile([C, 1], f32, tag="ps")
        nc.tensor.matmul(pS, expEgT, onescol, start=True, stop=True)
        recipL = sm.tile([C, 1], f32, tag="recipL")
        nc.vector.reciprocal(recipL, pS)
        # expEn[l, q] = expE[l, q] / S[l]
        expEn = wk.tile([C, C], f32, tag="expEn")
        nc.vector.tensor_scalar(out=expEn, in0=expE, scalar1=recipL,
                                scalar2=None, op0=OP.mult)
        # scaleLR = [recipL | grv]
        scaleLR = sm.tile([128, 1], f32, tag="scaleLR")
        nc.vector.tensor_copy(scaleLR[0:64], recipL)
        nc.vector.tensor_copy(scaleLR[64:128], grv)

        # transposed attention outputs: aoT [u, kt, rows]  (rows = l | r)
        aoT = wk.tile([128, KT, 128], f32, tag="aoT")
        for ut in range(KT):
            pl = ps.tile([128, C], f32, tag="ps")
            nc.tensor.matmul(pl, v_sb[:, ut * 128:(ut + 1) * 128], expEgT,
                             start=True, stop=True)
            nc.vector.tensor_copy(aoT[:, ut, 0:64], pl)
            pr = ps.tile([128, C], f32, tag="ps")
            nc.tensor.matmul(pr, v_sb[:, ut * 128:(ut + 1) * 128], expEn,
                             start=True, stop=True)
            nc.vector.tensor_copy(aoT[:, ut, 64:128], pr)

        # beta + residual precombine (off critical path)
        bx_lr = cst.tile([128, T], f32, tag="bxlr")
        nc.vector.tensor_add(bx_lr, b_lr, xlr)

        # ---- proj + stacked LN1 + residual -> OUT_LR [128 rows, T] ----
        pp = ps.tile([128, T], f32, tag="ps")
        for kt in range(KT):
            nc.tensor.matmul(pp, aoT[:, kt], wpt[:, kt],
                             start=(kt == 0), stop=(kt == KT - 1))
        OUT_LR = wk.tile([128, T], f32, tag="OUTLR")
        # (pp * scaleLR) + bp   (row scale folds softmax denom / bias terms)
        nc.vector.scalar_tensor_tensor(out=OUT_LR, in0=pp, scalar=scaleLR,
                                       in1=bp_t, op0=OP.mult, op1=OP.add)
        stats = sm.tile([128, 6], f32, tag="stats1")
        nc.vector.bn_stats(out=stats, in_=OUT_LR)
        mv = sm.tile([128, 2], f32, tag="mv1")
        nc.vector.bn_aggr(out=mv, in_=stats)
        rstd = sm.tile([128, 1], f32, tag="rstd1")
        nc.scalar.activation(out=rstd, in_=mv[:, 1:2], func=AF.Sqrt,
                             bias=eps_t)
        nc.vector.reciprocal(rstd, rstd)
        nc.vector.tensor_scalar(out=OUT_LR, in0=OUT_LR, scalar1=mv[:, 0:1],
                                scalar2=rstd, op0=OP.subtract, op1=OP.mult)
        nc.vector.tensor_tensor(out=OUT_LR, in0=OUT_LR, in1=g_lr, op=OP.mult)
        nc.vector.tensor_tensor(out=OUT_LR, in0=OUT_LR, in1=bx_lr, op=OP.add)

        # ---- transposes for router/experts: oT32 [u, kt, rows] ----
        oT32 = wk.tile([128, KT, 128], f32, tag="oT32")
        for kt in range(KT):
            pt = ps.tile([128, 128], f32, tag="ps")
            nc.tensor.transpose(pt, OUT_LR[:, kt * 128:(kt + 1) * 128], ident)
            nc.vector.tensor_copy(oT32[:, kt], pt)
        # fp8 hi copy of activations (ACT engine)
        oA8h = wk.tile([128, KT, 128], f8, tag="oA8h")
        nc.scalar.copy(out=oA8h[:, 0], in_=oT32[:, 0])
        nc.scalar.copy(out=oA8h[:, 1], in_=oT32[:, 1])

        # ---- router -> sims -> top-2 membership masks ----
        pxp = ps.tile([EXP, C], f32, tag="ps")
        j = 0
        for side in range(2):
            for kt in range(KT):
                nc.tensor.matmul(pxp, wrt[:, side * KT + kt],
                                 oT32[:, kt, side * 64:(side + 1) * 64],
                                 start=(j == 0), stop=(j == 3))
                j += 1
        xpT = wk.tile([EXP, C], f32, tag="xpT")
        nc.vector.tensor_scalar(out=xpT, in0=pxp, scalar1=brp, scalar2=None,
                                op0=OP.add)
        # sims (unnormalized; top-2 membership is row-norm invariant)
        psim = ps.tile([C, C], f32, tag="ps")
        nc.tensor.matmul(psim, xpT, cent, start=True, stop=True)
        mx8 = sm.tile([C, 8], f32, tag="mx8")
        nc.vector.max(out=mx8, in_=psim)
        # R[c, e] = sim[c, e] >= second_max[c]
        Rcm = sm.tile([C, C], f32, tag="Rcm")
        nc.vector.tensor_scalar(out=Rcm, in0=psim, scalar1=mx8[:, 1:2],
                                scalar2=None, op0=OP.is_ge)
        pRT = ps.tile([C, C], f32, tag="ps")
        nc.tensor.transpose(pRT, Rcm, ident[0:C, 0:C])
        RTh = sm.tile([C, C], f16, tag="RTh")
        nc.vector.tensor_copy(RTh, pRT)
        RT255 = sm.tile([C, C], u8, tag="RT255")
        nc.vector.tensor_scalar(out=RT255, in0=pRT, scalar1=255.0,
                                scalar2=None, op0=OP.mult)

        # ---- mask byte replication via DRAM round-trip (scalar queue) ----
        rtd = dram.tile([C, C], u8)
        nc.scalar.dma_start(out=rtd[:], in_=RT255)
        rrep = wk.tile([128, C, C], u8, tag="rrep")
        rsrc = rtd[:]
        for ch in range(8):
            cs = slice(ch * 8, (ch + 1) * 8)
            src_ap = bass.AP(tensor=rsrc.tensor,
                             offset=rsrc.offset + ch * 8 * C,
                             ap=[[0, 128], [C, 8], [1, C]])
            nc.scalar.dma_start(out=rrep[:, cs], in_=src_ap)

        # ---- expert bias via RTh matmuls ----
        ps_moe = ps_moe_p.tile([128, T], f32, tag="psmoe")
        nc.tensor.matmul(ps_moe[0:C], RTh, beh, start=True, stop=False,
                         skip_group_check=True)
        nc.tensor.matmul(ps_moe[C:128], RTh, beh, start=True, stop=False,
                         skip_group_check=True)

        # lo residual of the fp8 acts (runs during the hi expert phase)
        dq = wk.tile([128, KT, 128], f32, tag="dq")
        nc.scalar.copy(out=dq[:, 0], in_=oA8h[:, 0])
        nc.scalar.copy(out=dq[:, 1], in_=oA8h[:, 1])
        lo32 = wk.tile([128, KT, 128], f32, tag="lo32")
        nc.vector.tensor_sub(lo32, oT32, dq)
        oA8l = wk.tile([128, KT, 128], f8, tag="oA8l")
        nc.scalar.copy(out=oA8l[:, 0], in_=lo32[:, 0])
        nc.scalar.copy(out=oA8l[:, 1], in_=lo32[:, 1])

        # ---- expert stage: u32-AND masking + fp8 DR matmuls, hi then lo ----
        EG = 8
        NG = C // EG

        def and_group(src8, g, tag):
            e0 = g * EG
            asc = asc_p.tile([128, EG, KT, 128], f8, tag=tag)
            out_ap = bass.AP(tensor=asc.tensor, offset=asc.offset,
                             ap=[list(asc.ap[0]), [KT * 128, EG],
                                 [1, KT * 128]]).bitcast(u32)
            in0 = bass.AP(tensor=src8.tensor, offset=src8.offset,
                          ap=[list(src8.ap[0]), [0, EG],
                              [1, KT * 128]]).bitcast(u32)
            rs = rrep[:, e0:e0 + EG]
            in1 = bass.AP(tensor=rs.tensor, offset=rs.offset,
                          ap=[list(rs.ap[0]), [C, EG], [0, KT * 2],
                              [1, C]]).bitcast(u32)
            nc.vector.tensor_tensor(out=out_ap, in0=in0, in1=in1,
                                    op=OP.bitwise_and)
            return asc

        for g in range(NG):
            asc = and_group(oA8h, g, "asch")
            for i in range(EG):
                e = g * EG + i
                nc.tensor.matmul(ps_moe, asc[:, i], we_sb[:, e],
                                 start=False, stop=False,
                                 perf_mode=DR, skip_group_check=True)
        for g in range(NG):
            asc = and_group(oA8l, g, "ascl")
            for i in range(EG):
                e = g * EG + i
                nc.tensor.matmul(ps_moe, asc[:, i], we_sb[:, e],
                                 start=False, stop=(e == C - 1),
                                 perf_mode=DR, skip_group_check=True)

        # ---- final stacked LN2 + residual ----
        obx = wk.tile([128, T], f32, tag="obx")
        nc.vector.tensor_tensor(out=obx, in0=OUT_LR, in1=mb_lr, op=OP.add)

        olr = wk.tile([128, T], f32, tag="olr")
        nc.vector.tensor_copy(olr, ps_moe)
        stats2 = sm.tile([128, 6], f32, tag="stats2")
        nc.vector.bn_stats(out=stats2, in_=olr)
        mv2 = sm.tile([128, 2], f32, tag="mv2")
        nc.vector.bn_aggr(out=mv2, in_=stats2)
        rstd2 = sm.tile([128, 1], f32, tag="rstd2")
        nc.scalar.activation(out=rstd2, in_=mv2[:, 1:2], func=AF.Sqrt,
                             bias=eps_t)
        nc.vector.reciprocal(rstd2, rstd2)
        nc.vector.tensor_scalar(out=olr, in0=olr, scalar1=mv2[:, 0:1],
                                scalar2=rstd2, op0=OP.subtract, op1=OP.mult)
        nc.vector.tensor_tensor(out=olr, in0=olr, in1=mg_lr, op=OP.mult)
        nc.vector.tensor_tensor(out=olr, in0=olr, in1=obx, op=OP.add)
        nc.sync.dma_start(out=olr_d.ap(), in_=olr)

    nc.compile()
    return nc


def _tile_t(w):
    t_in, n = w.shape
    return np.ascontiguousarray(w.reshape(t_in // 128, 128, n).transpose(1, 0, 2))


def _prep_in_maps(inputs):
    f = np.float32
    import ml_dtypes
    x_l, x_r = np.asarray(inputs["x_l"], f), np.asarray(inputs["x_r"], f)

    Wq = np.asarray(inputs["Wq"], f)
    Wk = np.asarray(inputs["Wk"], f)
    M = Wq.T @ Wk
    w1 = Wk.T @ np.asarray(inputs["bq"], f)

    cen = np.asarray(inputs["centers"], f)
    cenn = cen / np.maximum(np.linalg.norm(cen, axis=-1, keepdims=True), 1e-12)
    vecs = np.zeros((1, 10 * T), f)
    for i, n in enumerate(VEC_ROWS):
        src = {"bv": "bv", "bp": "bp", "agl": "ag_l", "agr": "ag_r",
               "abl": "ab_l", "abr": "ab_r", "mgl": "mg_l", "mgr": "mg_r",
               "mbl": "mb_l", "mbr": "mb_r"}[n]
        vecs[0, i * T:(i + 1) * T] = np.asarray(inputs[src], f)

    arrs = {
        "mt": _tile_t(M),
        "wvt": _tile_t(np.asarray(inputs["Wv"], f).T),
        "wpt": _tile_t(np.asarray(inputs["Wp"], f).T),
        "w1t": _tile_t(w1.reshape(T, 1)),
        "wrt": _tile_t(np.asarray(inputs["Wr"], f).T),
        "brp": np.asarray(inputs["br"], f).reshape(EXP, 1),
        "cent": np.ascontiguousarray(cenn.T),
        "ident": np.eye(128, dtype=f),
    }
    We = np.asarray(inputs["We"], f) * WE_SCALE
    WeTh = np.ascontiguousarray(
        We.transpose(0, 2, 1).reshape(C, KT, 128, T).transpose(2, 0, 1, 3)
    ).astype(ml_dtypes.float8_e4m3)
    beh = (np.asarray(inputs["be"], f) * WE_SCALE).astype(np.float16)

    def pack(spec, ncols, extra):
        blob = np.zeros((128, ncols), f)
        for name, parts, shape in spec:
            off, _, _ = BLOB_OFF[name]
            cols = int(np.prod(shape[1:]))
            a = extra[name] if name in extra else arrs[name]
            blob[0:parts, off:off + cols] = np.asarray(a, f).reshape(parts, cols)
        return blob

    blobB = pack(BLOB_B_SPEC, NB_COLS, {})
    in_maps = []
    for b in range(N_CORES):
        xtl = _tile_t(np.ascontiguousarray(x_l[b].T))
        xtr = _tile_t(np.ascontiguousarray(x_r[b].T))
        xlr = np.concatenate([x_l[b], x_r[b]], axis=0)
        blobA = pack(BLOB_A_SPEC, NA_COLS,
                     {"xtl": xtl, "xtr": xtr, "xlr": xlr})
        in_maps.append({"blobA": blobA, "blobB": blobB, "vecsd": vecs,
                        "weh": WeTh, "beh": beh})
    return in_maps


def kernel(**inputs) -> np.ndarray:
    from concourse.bass_utils import run_bass_kernel_spmd

    if "nc" not in _CACHE:
        _CACHE["nc"] = _build()
    nc = _CACHE["nc"]
    in_maps = _prep_in_maps(inputs)
    res = run_bass_kernel_spmd(nc, in_maps, list(range(N_CORES)))
    _CACHE["exec_time_ns"] = res.exec_time_ns
    olr = np.stack([res.results[b]["olr"] for b in range(N_CORES)])
    return np.stack([olr[:, 0:C, :], olr[:, C:128, :]]).astype(np.float32)


# revision 27
# speedup vs baseline: 1.1175x; 1.1175x over previous
"""Trainium2 Bass kernel for nn_BiDGNBlock (moe_routing).

Data-parallel over batch across 8 NeuronCores (no collectives). Each core
computes one batch element end-to-end.

Structure (v2):
  - Expert table We streamed as fp8e4m3 (x64 prescaled; final LN is
    scale-invariant) -> 4.2MB. Expert matmuls in DoubleRow fp8 perf mode.
    Activations split hi+lo fp8e4m3 (two DR passes) so only the weight
    quantization error remains (~3.6% RMS -> ~1.5e-2 final rel).
  - Attention computes transposed outputs directly (out.T = v.T-style
    matmuls against scaled expE), with the softmax denominator and the
    bias-fold factors applied as per-row scalings folded into the
    projection stage. Both sides stacked as 128 rows everywhere.
  - Router: top-2 membership is invariant to the xp row norms, so the
    whole normalization chain is skipped; masks come from
    sim >= second_max per channel (is_ge), transposed once on the PE.
  - Mask bytes (0x00/0xFF) replicated across partitions via a small DRAM
    round-trip on the scalar engine's DMA queue; applied to fp8 acts with
    uint32 bitwise-AND ops on the DVE.
  - Replicated LN vectors loaded with partition-broadcast DMA reads on the
    scalar queue (no PE/DVE cost, no host-side replication bytes).
Routing stays exact-fp32 end-to-end.
"""

import sys
import numpy as np

sys.path.insert(0, "/opt/trn_rl_repo")

N_CORES = 8
B, C, T = 8, 64, 256
EXP = 32
KT = T // 128
WE_SCALE = 64.0

_CACHE: dict = {}

BLOB_A0_SPEC = [
    ("xtl", 128, (128, KT, C)), ("mt", 128, (128, KT, T)),
]
BLOB_A_SPEC = [
    ("xtr", 128, (128, KT, C)), ("wvt", 128, (128, KT, T)),
    ("xlr", 128, (128, T)), ("ident", 128, (128, 128)),
    ("w1t", 128, (128, KT, 1)),
]
BLOB_B_SPEC = [
    ("wpt", 128, (128, KT, T)), ("wrt", 128, (128, 2 * KT, EXP)),
    ("brp", 32, (32, 1)), ("cent", 32, (32, C)),
]
VEC_ROWS = ["bv", "bp", "agl", "agr", "abl", "abr", "mgl", "mgr",
            "mbl", "mbr"]


def _blob_layout():
    off = {}
    na0 = 0
    for name, parts, shape in BLOB_A0_SPEC:
        cols = int(np.prod(shape[1:]))
        off[name] = (na0, parts, shape)
        na0 += cols
    na = 0
    for name, parts, shape in BLOB_A_SPEC:
        cols = int(np.prod(shape[1:]))
        off[name] = (na, parts, shape)
        na += cols
    nb = 0
    for name, parts, shape in BLOB_B_SPEC:
        cols = int(np.prod(shape[1:]))
        off[name] = (nb, parts, shape)
        nb += cols
    return off, na0, na, nb


BLOB_OFF, NA0_COLS, NA_COLS, NB_COLS = _blob_layout()


def _build():
    import concourse.bass as bass
    import concourse.mybir as mybir
    import concourse.tile as tile
    from concourse import bacc
    from contextlib import ExitStack

    dt = mybir.dt
    f32, f16, f8, u8 = dt.float32, dt.float16, dt.float8e4, dt.uint8
    u32 = dt.uint32
    AF = mybir.ActivationFunctionType
    OP = mybir.AluOpType
    DR = mybir.MatmulPerfMode.DoubleRow

    nc = bacc.Bacc("TRN2", target_bir_lowering=False, debug=False,
                   num_devices=N_CORES)

    def inp(name, shape, d=f32):
        return nc.dram_tensor(name, list(shape), d, kind="ExternalInput")

    blobA0_d = inp("blobA0", (128, NA0_COLS))
    blobA_d = inp("blobA", (128, NA_COLS))
    blobB_d = inp("blobB", (128, NB_COLS))
    vecs_d = inp("vecsd", (1, 10 * T))
    weh_d = inp("weh", (128, C, KT, T), f8)
    beh_d = inp("beh", (C, T), f16)

    olr_d = nc.dram_tensor("olr", [128, T], f32, kind="ExternalOutput")

    with tile.TileContext(nc) as tc, ExitStack() as ctx:
        cst = ctx.enter_context(tc.tile_pool(name="cst", bufs=1))
        wk = ctx.enter_context(tc.tile_pool(name="wk", bufs=2))
        sm = ctx.enter_context(tc.tile_pool(name="sm", bufs=2))
        asc_p = ctx.enter_context(tc.tile_pool(name="asc", bufs=3))
        ps = ctx.enter_context(tc.tile_pool(name="ps", bufs=3, space="PSUM"))
        ps_moe_p = ctx.enter_context(tc.tile_pool(name="psmoe", bufs=1,
                                                  space="PSUM"))
        dram = ctx.enter_context(tc.tile_pool(name="dram", bufs=1,
                                              space="DRAM"))

        # ---- input DMAs: attention-critical blobA0 first on sync queue ----
        blobA0 = cst.tile([128, NA0_COLS], f32, tag="blobA0")
        nc.sync.dma_start(out=blobA0, in_=blobA0_d.ap())
        blobA = cst.tile([128, NA_COLS], f32, tag="blobA")
        nc.sync.dma_start(out=blobA, in_=blobA_d.ap())
        vecs_sb = cst.tile([1, 10 * T], f32, tag="vecs")
        nc.sync.dma_start(out=vecs_sb, in_=vecs_d.ap())
        blobB = cst.tile([128, NB_COLS], f32, tag="blobB")
        nc.sync.dma_start(out=blobB, in_=blobB_d.ap())
        we_sb = cst.tile([128, C, KT, T], f8, tag="weh")
        wea = weh_d.ap()
        for ch in range(8):
            nc.sync.dma_start(out=we_sb[:, ch * 8:(ch + 1) * 8],
                              in_=wea[:, ch * 8:(ch + 1) * 8])
        # beh on the (otherwise idle) scalar queue
        beh = cst.tile([C, T], f16, tag="beh")
        nc.scalar.dma_start(out=beh, in_=beh_d.ap())

        # replicated LN vectors via gpsimd partition_broadcast (no DMA, no PE).
        # NOTE: on HW the broadcast only works with dst base partition 0, so
        # stacked l|r tiles broadcast the l-vector to all 128 rows and then
        # overwrite rows 64-127 with a Pool copy from an r-scratch tile.
        vscr = cst.tile([128, T], f32, tag="vscr")

        def vec_rep2(i_l, i_r, tag):
            t_sb = cst.tile([128, T], f32, tag=tag)
            nc.gpsimd.partition_broadcast(
                t_sb, vecs_sb[0:1, i_l * T:(i_l + 1) * T])
            if i_r != i_l:
                nc.gpsimd.partition_broadcast(
                    vscr, vecs_sb[0:1, i_r * T:(i_r + 1) * T])
                nc.gpsimd.tensor_copy(t_sb[64:128], vscr[64:128])
            return t_sb

        bp_t = vec_rep2(1, 1, "bp")
        g_lr = vec_rep2(2, 3, "glr")
        b_lr = vec_rep2(4, 5, "blr")
        mg_lr = vec_rep2(6, 7, "mglr")
        mb_lr = vec_rep2(8, 9, "mblr")
        bv_t = cst.tile([64, T], f32, tag="bv")
        nc.gpsimd.partition_broadcast(bv_t, vecs_sb[0:1, 0:T])

        def bview(blob, name):
            off, parts, shape = BLOB_OFF[name]
            cols = 1
            for s in shape[1:]:
                cols *= s
            v = blob[0:parts, off:off + cols]
            if len(shape) == 3:
                v = v.rearrange("p (a b) -> p a b", a=shape[1])
            return v

        xtl = bview(blobA0, "xtl")
        mt = bview(blobA0, "mt")
        xtr = bview(blobA, "xtr")
        wvt = bview(blobA, "wvt")
        xlr = bview(blobA, "xlr")
        ident = bview(blobA, "ident")
        w1t = bview(blobA, "w1t")
        wpt = bview(blobB, "wpt")
        wrt = bview(blobB, "wrt")
        brp = bview(blobB, "brp")
        cent = bview(blobB, "cent")

        eps_t = cst.tile([128, 1], f32, tag="eps")
        nc.vector.memset(eps_t, 1e-5)
        onescol = cst.tile([64, 1], f32, tag="onescol")
        nc.vector.memset(onescol, 1.0)

        # PE warm-up + ACT table preloads during the DMA window
        warm_p = ctx.enter_context(tc.tile_pool(name="warm", bufs=1,
                                                space="PSUM"))
        wsrc = cst.tile([128, 512], f16, tag="wsrc")
        nc.vector.memset(wsrc, 0.5)
        pw = warm_p.tile([128, 512], f32, tag="warm")
        for wi in range(6):
            nc.tensor.matmul(pw, wsrc[:, 0:128], wsrc,
                             start=True, stop=True, skip_group_check=True)
        # preload only the Exp table now; Sqrt is prefetched later via a
        # dummy activation so the Exp->Sqrt switch happens off-path exactly
        # once (table loads cost 1.28us on the ACT engine).
        wact = cst.tile([1, 32], f32, tag="wact")
        nc.vector.memset(wact, 1.0)
        nc.scalar.activation(out=wact, in_=wact, func=AF.Exp)

        # ---- attention ----
        # A.T = (x_l @ M).T  [u, kt, cq]
        AT = wk.tile([128, KT, C], f32, tag="AT")
        for uo in range(KT):
            p = ps.tile([128, C], f32, tag="ps")
            for kt in range(KT):
                nc.tensor.matmul(p, mt[:, kt, uo * 128:(uo + 1) * 128],
                                 xtl[:, kt], start=(kt == 0),
                                 stop=(kt == KT - 1))
            nc.vector.tensor_copy(AT[:, uo], p)
        # energy E = A @ x_r.T
        pe_ = ps.tile([C, C], f32, tag="ps")
        for kt in range(KT):
            nc.tensor.matmul(pe_, AT[:, kt], xtr[:, kt],
                             start=(kt == 0), stop=(kt == KT - 1))
        # expE = exp(E/16)  (|E|/16 small enough to skip max-subtract)
        expE = wk.tile([C, C], f32, tag="expE")
        nc.scalar.activation(out=expE, in_=pe_, func=AF.Exp, scale=1.0 / 16.0)
        # rv.T = x_r @ (Wk.T bq); grv = exp(rv/16)  (bias fold, g-column)
        prv = ps.tile([C, 1], f32, tag="ps")
        for kt in range(KT):
            nc.tensor.matmul(prv, xtr[:, kt], w1t[:, kt],
                             start=(kt == 0), stop=(kt == KT - 1))
        grv = sm.tile([C, 1], f32, tag="grv")
        nc.scalar.activation(out=grv, in_=prv, func=AF.Exp, scale=1.0 / 16.0)
        # prefetch the Sqrt table while the DVE runs the LN1 stats
        nc.scalar.activation(out=wact, in_=wact, func=AF.Sqrt)
        # v = (x_l - x_r) @ Wv.T + bv   [ck, u]
        xdt = wk.tile([128, KT, C], f32, tag="xdt")
        nc.vector.tensor_sub(xdt, xtl, xtr)
        pv = ps.tile([C, T], f32, tag="ps")
        for kt in range(KT):
            nc.tensor.matmul(pv, xdt[:, kt], wvt[:, kt],
                             start=(kt == 0), stop=(kt == KT - 1))
        v_sb = wk.tile([C, T], f32, tag="v")
        nc.vector.tensor_tensor(out=v_sb, in0=pv, in1=bv_t, op=OP.add)

        # expEg_T[ck, cq] = expE[cq, ck] * g[ck]  (transpose + g-fold)
        pet = ps.tile([C, C], f32, tag="ps")
        nc.tensor.transpose(pet, expE, ident[0:C, 0:C])
        expEgT = wk.tile([C, C], f32, tag="expEgT")
        nc.vector.tensor_scalar(out=expEgT, in0=pet, scalar1=grv,
                                scalar2=None, op0=OP.mult)
        # S[cq] = col sums of expEg_T;  recipL = 1/S
        pS = ps.tile([C, 1], f32, tag="ps")
        nc.tensor.matmul(pS, expEgT, onescol, start=True, stop=True)
        recipL = sm.tile([C, 1], f32, tag="recipL")
        nc.vector.reciprocal(recipL, pS)
        # expEn[l, q] = expE[l, q] / S[l]
        expEn = wk.tile([C, C], f32, tag="expEn")
        nc.vector.tensor_scalar(out=expEn, in0=expE, scalar1=recipL,
                                scalar2=None, op0=OP.mult)
        # scaleLR = [recipL | grv]
        scaleLR = sm.tile([128, 1], f32, tag="scaleLR")
        nc.vector.tensor_copy(scaleLR[0:64], recipL)
        nc.vector.tensor_copy(scaleLR[64:128], grv)

        # transposed attention outputs: aoT [u, kt, rows]  (rows = l | r)
        aoT = wk.tile([128, KT, 128], f32, tag="aoT")
        for ut in range(KT):
            pl = ps.tile([128, C], f32, tag="ps")
            nc.tensor.matmul(pl, v_sb[:, ut * 128:(ut + 1) * 128], expEgT,
                             start=True, stop=True)
            nc.vector.tensor_copy(aoT[:, ut, 0:64], pl)
            pr = ps.tile([128, C], f32, tag="ps")
            nc.tensor.matmul(pr, v_sb[:, ut * 128:(ut + 1) * 128], expEn,
                             start=True, stop=True)
            nc.vector.tensor_copy(aoT[:, ut, 64:128], pr)

        # beta + residual precombine (off critical path)
        bx_lr = cst.tile([128, T], f32, tag="bxlr")
        nc.vector.tensor_add(bx_lr, b_lr, xlr)

        # ---- proj + stacked LN1 + residual -> OUT_LR [128 rows, T] ----
        pp = ps.tile([128, T], f32, tag="ps")
        for kt in range(KT):
            nc.tensor.matmul(pp, aoT[:, kt], wpt[:, kt],
                             start=(kt == 0), stop=(kt == KT - 1))
        OUT_LR = wk.tile([128, T], f32, tag="OUTLR")
        # (pp * scaleLR) + bp   (row scale folds softmax denom / bias terms)
        nc.vector.scalar_tensor_tensor(out=OUT_LR, in0=pp, scalar=scaleLR,
                                       in1=bp_t, op0=OP.mult, op1=OP.add)
        stats = sm.tile([128, 6], f32, tag="stats1")
        nc.vector.bn_stats(out=stats, in_=OUT_LR)
        mv = sm.tile([128, 2], f32, tag="mv1")
        nc.vector.bn_aggr(out=mv, in_=stats)
        rstd = sm.tile([128, 1], f32, tag="rstd1")
        nc.scalar.activation(out=rstd, in_=mv[:, 1:2], func=AF.Sqrt,
                             bias=eps_t)
        nc.vector.reciprocal(rstd, rstd)
        nc.vector.tensor_scalar(out=OUT_LR, in0=OUT_LR, scalar1=mv[:, 0:1],
                                scalar2=rstd, op0=OP.subtract, op1=OP.mult)
        nc.vector.tensor_tensor(out=OUT_LR, in0=OUT_LR, in1=g_lr, op=OP.mult)
        nc.vector.tensor_tensor(out=OUT_LR, in0=OUT_LR, in1=bx_lr, op=OP.add)

        # ---- transposes for router/experts: oT32 [u, kt, rows] ----
        oT32 = wk.tile([128, KT, 128], f32, tag="oT32")
        for kt in range(KT):
            pt = ps.tile([128, 128], f32, tag="ps")
            nc.tensor.transpose(pt, OUT_LR[:, kt * 128:(kt + 1) * 128], ident)
            nc.vector.tensor_copy(oT32[:, kt], pt)
        # fp8 hi copy of activations (gpsimd: no ACT tables, Pool is idle)
        # oA8 holds [hi | lo] halves: [128, 2(hl), KT, 128]
        oA8 = wk.tile([128, 2, KT, 128], f8, tag="oA8")
        nc.gpsimd.tensor_copy(oA8[:, 0], oT32)

        # ---- router -> sims -> top-2 membership masks ----
        pxp = ps.tile([EXP, C], f32, tag="ps")
        j = 0
        for side in range(2):
            for kt in range(KT):
                nc.tensor.matmul(pxp, wrt[:, side * KT + kt],
                                 oT32[:, kt, side * 64:(side + 1) * 64],
                                 start=(j == 0), stop=(j == 3))
                j += 1
        xpT = wk.tile([EXP, C], f32, tag="xpT")
        nc.vector.tensor_scalar(out=xpT, in0=pxp, scalar1=brp, scalar2=None,
                                op0=OP.add)
        # sims (unnormalized; top-2 membership is row-norm invariant)
        psim = ps.tile([C, C], f32, tag="ps")
        nc.tensor.matmul(psim, xpT, cent, start=True, stop=True)
        mx8 = sm.tile([C, 8], f32, tag="mx8")
        nc.vector.max(out=mx8, in_=psim)
        # R[c, e] = sim[c, e] >= second_max[c]
        Rcm = sm.tile([C, C], f32, tag="Rcm")
        nc.vector.tensor_scalar(out=Rcm, in0=psim, scalar1=mx8[:, 1:2],
                                scalar2=None, op0=OP.is_ge)
        pRT = ps.tile([C, C], f32, tag="ps")
        nc.tensor.transpose(pRT, Rcm, ident[0:C, 0:C])
        RTh = sm.tile([C, C], f16, tag="RTh")
        nc.vector.tensor_copy(RTh, pRT)
        RT255 = sm.tile([C, C], u8, tag="RT255")
        nc.vector.tensor_scalar(out=RT255, in0=pRT, scalar1=255.0,
                                scalar2=None, op0=OP.mult)

        # ---- mask byte replication via DRAM round-trip (scalar queue) ----
        # one store + one load with contiguous 4KB-per-partition reads
        rtd = dram.tile([C, C], u8)
        nc.scalar.dma_start(out=rtd[:], in_=RT255)
        rrep = wk.tile([128, C, C], u8, tag="rrep")
        rsrc = rtd[:]
        src_ap = bass.AP(tensor=rsrc.tensor, offset=rsrc.offset,
                         ap=[[0, 128], [1, C * C]])
        nc.scalar.dma_start(out=rrep, in_=src_ap)

        # ---- expert bias via RTh matmuls ----
        ps_moe = ps_moe_p.tile([128, T], f32, tag="psmoe")
        nc.tensor.matmul(ps_moe[0:C], RTh, beh, start=True, stop=False,
                         skip_group_check=True)
        nc.tensor.matmul(ps_moe[C:128], RTh, beh, start=True, stop=False,
                         skip_group_check=True)

        # lo residual of the fp8 acts (gpsimd casts, DVE subtract)
        dq = wk.tile([128, KT, 128], f32, tag="dq")
        nc.gpsimd.tensor_copy(dq, oA8[:, 0])
        lo32 = wk.tile([128, KT, 128], f32, tag="lo32")
        nc.vector.tensor_sub(lo32, oT32, dq)
        nc.gpsimd.tensor_copy(oA8[:, 1], lo32)

        # ---- expert stage: u32-AND masking + fp8 DR matmuls (hi+lo) ----
        EG = 8
        NG = C // EG
        for g in range(NG):
            e0 = g * EG
            # asc [128, EG, 2(hl), KT, 128]: one AND covers hi and lo
            asc = asc_p.tile([128, EG, 2, KT, 128], f8, tag="asc")
            out_ap = bass.AP(tensor=asc.tensor, offset=asc.offset,
                             ap=[list(asc.ap[0]), [2 * KT * 128, EG],
                                 [1, 2 * KT * 128]]).bitcast(u32)
            in0 = bass.AP(tensor=oA8.tensor, offset=oA8.offset,
                          ap=[list(oA8.ap[0]), [0, EG],
                              [1, 2 * KT * 128]]).bitcast(u32)
            rs = rrep[:, e0:e0 + EG]
            in1 = bass.AP(tensor=rs.tensor, offset=rs.offset,
                          ap=[list(rs.ap[0]), [C, EG], [0, 2 * KT * 2],
                              [1, C]]).bitcast(u32)
            nc.vector.tensor_tensor(out=out_ap, in0=in0, in1=in1,
                                    op=OP.bitwise_and)
            for i in range(EG):
                e = e0 + i
                for hl in range(2):
                    nc.tensor.matmul(ps_moe, asc[:, i, hl], we_sb[:, e],
                                     start=False,
                                     stop=(e == C - 1 and hl == 1),
                                     perf_mode=DR, skip_group_check=True)

        # ---- final stacked LN2 + residual ----
        obx = wk.tile([128, T], f32, tag="obx")
        nc.vector.tensor_tensor(out=obx, in0=OUT_LR, in1=mb_lr, op=OP.add)

        olr = wk.tile([128, T], f32, tag="olr")
        nc.vector.tensor_copy(olr, ps_moe)
        stats2 = sm.tile([128, 6], f32, tag="stats2")
        nc.vector.bn_stats(out=stats2, in_=olr)
        mv2 = sm.tile([128, 2], f32, tag="mv2")
        nc.vector.bn_aggr(out=mv2, in_=stats2)
        rstd2 = sm.tile([128, 1], f32, tag="rstd2")
        nc.scalar.activation(out=rstd2, in_=mv2[:, 1:2], func=AF.Sqrt,
                             bias=eps_t)
        nc.vector.reciprocal(rstd2, rstd2)
        nc.vector.tensor_scalar(out=olr, in0=olr, scalar1=mv2[:, 0:1],
                                scalar2=rstd2, op0=OP.subtract, op1=OP.mult)
        nc.vector.tensor_tensor(out=olr, in0=olr, in1=mg_lr, op=OP.mult)
        nc.vector.tensor_tensor(out=olr, in0=olr, in1=obx, op=OP.add)
        nc.sync.dma_start(out=olr_d.ap(), in_=olr)

    nc.compile()
    return nc


def _tile_t(w):
    t_in, n = w.shape
    return np.ascontiguousarray(w.reshape(t_in // 128, 128, n).transpose(1, 0, 2))


def _prep_in_maps(inputs):
    f = np.float32
    import ml_dtypes
    x_l, x_r = np.asarray(inputs["x_l"], f), np.asarray(inputs["x_r"], f)

    Wq = np.asarray(inputs["Wq"], f)
    Wk = np.asarray(inputs["Wk"], f)
    M = Wq.T @ Wk
    w1 = Wk.T @ np.asarray(inputs["bq"], f)

    cen = np.asarray(inputs["centers"], f)
    cenn = cen / np.maximum(np.linalg.norm(cen, axis=-1, keepdims=True), 1e-12)
    vecs = np.zeros((1, 10 * T), f)
    for i, n in enumerate(VEC_ROWS):
        src = {"bv": "bv", "bp": "bp", "agl": "ag_l", "agr": "ag_r",
               "abl": "ab_l", "abr": "ab_r", "mgl": "mg_l", "mgr": "mg_r",
               "mbl": "mb_l", "mbr": "mb_r"}[n]
        vecs[0, i * T:(i + 1) * T] = np.asarray(inputs[src], f)

    arrs = {
        "mt": _tile_t(M),
        "wvt": _tile_t(np.asarray(inputs["Wv"], f).T),
        "wpt": _tile_t(np.asarray(inputs["Wp"], f).T),
        "w1t": _tile_t(w1.reshape(T, 1)),
        "wrt": _tile_t(np.asarray(inputs["Wr"], f).T),
        "brp": np.asarray(inputs["br"], f).reshape(EXP, 1),
        "cent": np.ascontiguousarray(cenn.T),
        "ident": np.eye(128, dtype=f),
    }
    We = np.asarray(inputs["We"], f) * WE_SCALE
    WeTh = np.ascontiguousarray(
        We.transpose(0, 2, 1).reshape(C, KT, 128, T).transpose(2, 0, 1, 3)
    ).astype(ml_dtypes.float8_e4m3)
    beh = (np.asarray(inputs["be"], f) * WE_SCALE).astype(np.float16)

    def pack(spec, ncols, extra):
        blob = np.zeros((128, ncols), f)
        for name, parts, shape in spec:
            off, _, _ = BLOB_OFF[name]
            cols = int(np.prod(shape[1:]))
            a = extra[name] if name in extra else arrs[name]
            blob[0:parts, off:off + cols] = np.asarray(a, f).reshape(parts, cols)
        return blob

    blobB = pack(BLOB_B_SPEC, NB_COLS, {})
    in_maps = []
    for b in range(N_CORES):
        xtl = _tile_t(np.ascontiguousarray(x_l[b].T))
        xtr = _tile_t(np.ascontiguousarray(x_r[b].T))
        xlr = np.concatenate([x_l[b], x_r[b]], axis=0)
        blobA0 = pack(BLOB_A0_SPEC, NA0_COLS, {"xtl": xtl})
        blobA = pack(BLOB_A_SPEC, NA_COLS, {"xtr": xtr, "xlr": xlr})
        in_maps.append({"blobA0": blobA0, "blobA": blobA, "blobB": blobB,
                        "vecsd": vecs, "weh": WeTh, "beh": beh})
    return in_maps


def kernel(**inputs) -> np.ndarray:
    from concourse.bass_utils import run_bass_kernel_spmd

    if "nc" not in _CACHE:
        _CACHE["nc"] = _build()
    nc = _CACHE["nc"]
    in_maps = _prep_in_maps(inputs)
    res = run_bass_kernel_spmd(nc, in_maps, list(range(N_CORES)))
    _CACHE["exec_time_ns"] = res.exec_time_ns
    olr = np.stack([res.results[b]["olr"] for b in range(N_CORES)])
    return np.stack([olr[:, 0:C, :], olr[:, C:128, :]]).astype(np.float32)


# revision 30
# speedup vs baseline: 1.1811x; 1.0570x over previous
"""Trainium2 Bass kernel for nn_BiDGNBlock (moe_routing).

Data-parallel over batch across 8 NeuronCores (no collectives). Each core
computes one batch element end-to-end.

Structure (v2):
  - Expert table We streamed as fp8e4m3 (x64 prescaled; final LN is
    scale-invariant) -> 4.2MB. Expert matmuls in DoubleRow fp8 perf mode.
    Activations split hi+lo fp8e4m3 (two DR passes) so only the weight
    quantization error remains (~3.6% RMS -> ~1.5e-2 final rel).
  - Attention computes transposed outputs directly (out.T = v.T-style
    matmuls against scaled expE), with the softmax denominator and the
    bias-fold factors applied as per-row scalings folded into the
    projection stage. Both sides stacked as 128 rows everywhere.
  - Router: top-2 membership is invariant to the xp row norms, so the
    whole normalization chain is skipped; masks come from
    sim >= second_max per channel (is_ge), transposed once on the PE.
  - Mask bytes (0x00/0xFF) replicated across partitions via a small DRAM
    round-trip on the scalar engine's DMA queue; applied to fp8 acts with
    uint32 bitwise-AND ops on the DVE.
  - Replicated LN vectors loaded with partition-broadcast DMA reads on the
    scalar queue (no PE/DVE cost, no host-side replication bytes).
Routing stays exact-fp32 end-to-end.
"""

import sys
import numpy as np

sys.path.insert(0, "/opt/trn_rl_repo")

N_CORES = 8
B, C, T = 8, 64, 256
EXP = 32
KT = T // 128
WE_SCALE = 64.0

_CACHE: dict = {}

BLOB_A0_SPEC = [
    ("xtl", 128, (128, KT, C)), ("mt", 128, (128, KT, T)),
]
BLOB_A_SPEC = [
    ("xtr", 128, (128, KT, C)), ("wvt", 128, (128, KT, T)),
    ("xlr", 128, (128, T)), ("ident", 128, (128, 128)),
    ("w1t", 128, (128, KT, 1)),
]
BLOB_B_SPEC = [
    ("wpt", 128, (128, KT, T)), ("wrt", 128, (128, 2 * KT, EXP)),
    ("brp", 32, (32, 1)), ("cent", 32, (32, C)),
]
VEC_ROWS = ["bv", "bp", "agl", "agr", "abl", "abr", "mgl", "mgr",
            "mbl", "mbr"]


def _blob_layout():
    off = {}
    na0 = 0
    for name, parts, shape in BLOB_A0_SPEC:
        cols = int(np.prod(shape[1:]))
        off[name] = (na0, parts, shape)
        na0 += cols
    na = 0
    for name, parts, shape in BLOB_A_SPEC:
        cols = int(np.prod(shape[1:]))
        off[name] = (na, parts, shape)
        na += cols
    nb = 0
    for name, parts, shape in BLOB_B_SPEC:
        cols = int(np.prod(shape[1:]))
        off[name] = (nb, parts, shape)
        nb += cols
    return off, na0, na, nb


BLOB_OFF, NA0_COLS, NA_COLS, NB_COLS = _blob_layout()


def _build():
    import concourse.bass as bass
    import concourse.mybir as mybir
    import concourse.tile as tile
    from concourse import bacc
    from contextlib import ExitStack

    dt = mybir.dt
    f32, f16, f8, u8 = dt.float32, dt.float16, dt.float8e4, dt.uint8
    u32 = dt.uint32
    AF = mybir.ActivationFunctionType
    OP = mybir.AluOpType
    DR = mybir.MatmulPerfMode.DoubleRow

    nc = bacc.Bacc("TRN2", target_bir_lowering=False, debug=False,
                   num_devices=N_CORES)

    def inp(name, shape, d=f32):
        return nc.dram_tensor(name, list(shape), d, kind="ExternalInput")

    blobA0_d = inp("blobA0", (128, NA0_COLS))
    blobA_d = inp("blobA", (128, NA_COLS))
    blobB_d = inp("blobB", (128, NB_COLS))
    vecs_d = inp("vecsd", (1, 10 * T))
    weh_d = inp("weh", (128, C, KT, T), f8)
    beh_d = inp("beh", (C, T), f16)

    olr_d = nc.dram_tensor("olr", [128, T], f32, kind="ExternalOutput")

    with tile.TileContext(nc) as tc, ExitStack() as ctx:
        cst = ctx.enter_context(tc.tile_pool(name="cst", bufs=1))
        wk = ctx.enter_context(tc.tile_pool(name="wk", bufs=2))
        sm = ctx.enter_context(tc.tile_pool(name="sm", bufs=2))
        asc_p = ctx.enter_context(tc.tile_pool(name="asc", bufs=3))
        ps = ctx.enter_context(tc.tile_pool(name="ps", bufs=3, space="PSUM"))
        ps_moe_p = ctx.enter_context(tc.tile_pool(name="psmoe", bufs=1,
                                                  space="PSUM"))
        dram = ctx.enter_context(tc.tile_pool(name="dram", bufs=1,
                                              space="DRAM"))

        # ---- input DMAs: attention-critical blobA0 first on sync queue ----
        blobA0 = cst.tile([128, NA0_COLS], f32, tag="blobA0")
        nc.sync.dma_start(out=blobA0, in_=blobA0_d.ap())
        blobA = cst.tile([128, NA_COLS], f32, tag="blobA")
        nc.sync.dma_start(out=blobA, in_=blobA_d.ap())
        blobB = cst.tile([128, NB_COLS], f32, tag="blobB")
        nc.sync.dma_start(out=blobB, in_=blobB_d.ap())
        we_sb = cst.tile([128, C, KT, T], f8, tag="weh")
        wea = weh_d.ap()
        for ch in range(8):
            nc.sync.dma_start(out=we_sb[:, ch * 8:(ch + 1) * 8],
                              in_=wea[:, ch * 8:(ch + 1) * 8])
        # small loads on the (otherwise idle) scalar queue
        vecs_sb = cst.tile([1, 10 * T], f32, tag="vecs")
        nc.scalar.dma_start(out=vecs_sb, in_=vecs_d.ap())
        beh = cst.tile([C, T], f16, tag="beh")
        nc.scalar.dma_start(out=beh, in_=beh_d.ap())

        # replicated LN vectors via gpsimd partition_broadcast (no DMA, no PE).
        # NOTE: on HW the broadcast only works with dst base partition 0, so
        # stacked l|r tiles broadcast the l-vector to all 128 rows and then
        # overwrite rows 64-127 with a Pool copy from an r-scratch tile.
        vscr = cst.tile([128, T], f32, tag="vscr")

        def vec_rep2(i_l, i_r, tag):
            t_sb = cst.tile([128, T], f32, tag=tag)
            nc.gpsimd.partition_broadcast(
                t_sb, vecs_sb[0:1, i_l * T:(i_l + 1) * T])
            if i_r != i_l:
                nc.gpsimd.partition_broadcast(
                    vscr, vecs_sb[0:1, i_r * T:(i_r + 1) * T])
                nc.gpsimd.tensor_copy(t_sb[64:128], vscr[64:128])
            return t_sb

        # order by consumer time: bv gates v (early), bp/g/b gate LN1,
        # mg/mb only gate LN2 (late)
        bv_t = cst.tile([64, T], f32, tag="bv")
        nc.gpsimd.partition_broadcast(bv_t, vecs_sb[0:1, 0:T])
        bp_t = vec_rep2(1, 1, "bp")
        g_lr = vec_rep2(2, 3, "glr")
        b_lr = vec_rep2(4, 5, "blr")
        mg_lr = vec_rep2(6, 7, "mglr")
        mb_lr = vec_rep2(8, 9, "mblr")

        def bview(blob, name):
            off, parts, shape = BLOB_OFF[name]
            cols = 1
            for s in shape[1:]:
                cols *= s
            v = blob[0:parts, off:off + cols]
            if len(shape) == 3:
                v = v.rearrange("p (a b) -> p a b", a=shape[1])
            return v

        xtl = bview(blobA0, "xtl")
        mt = bview(blobA0, "mt")
        xtr = bview(blobA, "xtr")
        wvt = bview(blobA, "wvt")
        xlr = bview(blobA, "xlr")
        ident = bview(blobA, "ident")
        w1t = bview(blobA, "w1t")
        wpt = bview(blobB, "wpt")
        wrt = bview(blobB, "wrt")
        brp = bview(blobB, "brp")
        cent = bview(blobB, "cent")

        eps_t = cst.tile([128, 1], f32, tag="eps")
        nc.vector.memset(eps_t, 1e-5)
        onescol = cst.tile([64, 1], f32, tag="onescol")
        nc.vector.memset(onescol, 1.0)

        # PE warm-up + ACT table preloads during the DMA window
        warm_p = ctx.enter_context(tc.tile_pool(name="warm", bufs=1,
                                                space="PSUM"))
        wsrc = cst.tile([128, 512], f16, tag="wsrc")
        nc.vector.memset(wsrc, 0.5)
        pw = warm_p.tile([128, 512], f32, tag="warm")
        for wi in range(6):
            nc.tensor.matmul(pw, wsrc[:, 0:128], wsrc,
                             start=True, stop=True, skip_group_check=True)
        # preload only the Exp table now; Sqrt is prefetched later via a
        # dummy activation so the Exp->Sqrt switch happens off-path exactly
        # once (table loads cost 1.28us on the ACT engine).
        wact = cst.tile([1, 32], f32, tag="wact")
        nc.vector.memset(wact, 1.0)
        nc.scalar.activation(out=wact, in_=wact, func=AF.Exp)

        # ---- attention ----
        # A.T = (x_l @ M).T  [u, kt, cq]
        AT = wk.tile([128, KT, C], f32, tag="AT")
        for uo in range(KT):
            p = ps.tile([128, C], f32, tag="ps")
            for kt in range(KT):
                nc.tensor.matmul(p, mt[:, kt, uo * 128:(uo + 1) * 128],
                                 xtl[:, kt], start=(kt == 0),
                                 stop=(kt == KT - 1))
            nc.vector.tensor_copy(AT[:, uo], p)
        # energy E = A @ x_r.T
        pe_ = ps.tile([C, C], f32, tag="ps")
        for kt in range(KT):
            nc.tensor.matmul(pe_, AT[:, kt], xtr[:, kt],
                             start=(kt == 0), stop=(kt == KT - 1))
        # expE = exp(E/16)  (|E|/16 small enough to skip max-subtract)
        expE = wk.tile([C, C], f32, tag="expE")
        nc.scalar.activation(out=expE, in_=pe_, func=AF.Exp, scale=1.0 / 16.0)
        # rv.T = x_r @ (Wk.T bq); grv = exp(rv/16)  (bias fold, g-column)
        prv = ps.tile([C, 1], f32, tag="ps")
        for kt in range(KT):
            nc.tensor.matmul(prv, xtr[:, kt], w1t[:, kt],
                             start=(kt == 0), stop=(kt == KT - 1))
        grv = sm.tile([C, 1], f32, tag="grv")
        nc.scalar.activation(out=grv, in_=prv, func=AF.Exp, scale=1.0 / 16.0)
        # prefetch the Sqrt table while the DVE runs the LN1 stats
        nc.scalar.activation(out=wact, in_=wact, func=AF.Sqrt)
        # v = (x_l - x_r) @ Wv.T + bv   [ck, u]
        xdt = wk.tile([128, KT, C], f32, tag="xdt")
        nc.vector.tensor_sub(xdt, xtl, xtr)
        pv = ps.tile([C, T], f32, tag="ps")
        for kt in range(KT):
            nc.tensor.matmul(pv, xdt[:, kt], wvt[:, kt],
                             start=(kt == 0), stop=(kt == KT - 1))
        v_sb = wk.tile([C, T], f32, tag="v")
        nc.vector.tensor_tensor(out=v_sb, in0=pv, in1=bv_t, op=OP.add)

        # expEg_T[ck, cq] = expE[cq, ck] * g[ck]  (transpose + g-fold)
        pet = ps.tile([C, C], f32, tag="ps")
        nc.tensor.transpose(pet, expE, ident[0:C, 0:C])
        expEgT = wk.tile([C, C], f32, tag="expEgT")
        nc.vector.tensor_scalar(out=expEgT, in0=pet, scalar1=grv,
                                scalar2=None, op0=OP.mult)
        # S[cq] = col sums of expEg_T;  recipL = 1/S
        pS = ps.tile([C, 1], f32, tag="ps")
        nc.tensor.matmul(pS, expEgT, onescol, start=True, stop=True)
        recipL = sm.tile([C, 1], f32, tag="recipL")
        nc.vector.reciprocal(recipL, pS)
        # expEn[l, q] = expE[l, q] / S[l]
        expEn = wk.tile([C, C], f32, tag="expEn")
        nc.vector.tensor_scalar(out=expEn, in0=expE, scalar1=recipL,
                                scalar2=None, op0=OP.mult)
        # scaleLR = [recipL | grv]
        scaleLR = sm.tile([128, 1], f32, tag="scaleLR")
        nc.vector.tensor_copy(scaleLR[0:64], recipL)
        nc.vector.tensor_copy(scaleLR[64:128], grv)

        # transposed attention outputs: aoT [u, kt, rows]  (rows = l | r)
        aoT = wk.tile([128, KT, 128], f32, tag="aoT")
        for ut in range(KT):
            pl = ps.tile([128, C], f32, tag="ps")
            nc.tensor.matmul(pl, v_sb[:, ut * 128:(ut + 1) * 128], expEgT,
                             start=True, stop=True)
            nc.vector.tensor_copy(aoT[:, ut, 0:64], pl)
            pr = ps.tile([128, C], f32, tag="ps")
            nc.tensor.matmul(pr, v_sb[:, ut * 128:(ut + 1) * 128], expEn,
                             start=True, stop=True)
            nc.vector.tensor_copy(aoT[:, ut, 64:128], pr)

        # beta + residual precombine (off critical path)
        bx_lr = cst.tile([128, T], f32, tag="bxlr")
        nc.vector.tensor_add(bx_lr, b_lr, xlr)

        # ---- proj + stacked LN1 + residual -> OUT_LR [128 rows, T] ----
        pp = ps.tile([128, T], f32, tag="ps")
        for kt in range(KT):
            nc.tensor.matmul(pp, aoT[:, kt], wpt[:, kt],
                             start=(kt == 0), stop=(kt == KT - 1))
        OUT_LR = wk.tile([128, T], f32, tag="OUTLR")
        # (pp * scaleLR) + bp   (row scale folds softmax denom / bias terms)
        nc.vector.scalar_tensor_tensor(out=OUT_LR, in0=pp, scalar=scaleLR,
                                       in1=bp_t, op0=OP.mult, op1=OP.add)
        stats = sm.tile([128, 6], f32, tag="stats1")
        nc.vector.bn_stats(out=stats, in_=OUT_LR)
        mv = sm.tile([128, 2], f32, tag="mv1")
        nc.vector.bn_aggr(out=mv, in_=stats)
        rstd = sm.tile([128, 1], f32, tag="rstd1")
        nc.scalar.activation(out=rstd, in_=mv[:, 1:2], func=AF.Sqrt,
                             bias=eps_t)
        nc.vector.reciprocal(rstd, rstd)
        nc.vector.tensor_scalar(out=OUT_LR, in0=OUT_LR, scalar1=mv[:, 0:1],
                                scalar2=rstd, op0=OP.subtract, op1=OP.mult)
        nc.vector.tensor_tensor(out=OUT_LR, in0=OUT_LR, in1=g_lr, op=OP.mult)
        nc.vector.tensor_tensor(out=OUT_LR, in0=OUT_LR, in1=bx_lr, op=OP.add)

        # ---- transposes for router/experts: oT32 [u, kt, rows] ----
        oT32 = wk.tile([128, KT, 128], f32, tag="oT32")
        for kt in range(KT):
            pt = ps.tile([128, 128], f32, tag="ps")
            nc.tensor.transpose(pt, OUT_LR[:, kt * 128:(kt + 1) * 128], ident)
            nc.vector.tensor_copy(oT32[:, kt], pt)
        # fp8 hi copy of activations (gpsimd: no ACT tables, Pool is idle)
        # oA8 holds [hi | lo] halves: [128, 2(hl), KT, 128]
        oA8 = wk.tile([128, 2, KT, 128], f8, tag="oA8")
        nc.gpsimd.tensor_copy(oA8[:, 0], oT32)

        # ---- router -> sims -> top-2 membership masks ----
        pxp = ps.tile([EXP, C], f32, tag="ps")
        j = 0
        for side in range(2):
            for kt in range(KT):
                nc.tensor.matmul(pxp, wrt[:, side * KT + kt],
                                 oT32[:, kt, side * 64:(side + 1) * 64],
                                 start=(j == 0), stop=(j == 3))
                j += 1
        xpT = wk.tile([EXP, C], f32, tag="xpT")
        nc.vector.tensor_scalar(out=xpT, in0=pxp, scalar1=brp, scalar2=None,
                                op0=OP.add)
        # sims (unnormalized; top-2 membership is row-norm invariant)
        psim = ps.tile([C, C], f32, tag="ps")
        nc.tensor.matmul(psim, xpT, cent, start=True, stop=True)
        mx8 = sm.tile([C, 8], f32, tag="mx8")
        nc.vector.max(out=mx8, in_=psim)
        # R[c, e] = sim[c, e] >= second_max[c]
        Rcm = sm.tile([C, C], f32, tag="Rcm")
        nc.vector.tensor_scalar(out=Rcm, in0=psim, scalar1=mx8[:, 1:2],
                                scalar2=None, op0=OP.is_ge)
        pRT = ps.tile([C, C], f32, tag="ps")
        nc.tensor.transpose(pRT, Rcm, ident[0:C, 0:C])
        RTh = sm.tile([C, C], f16, tag="RTh")
        nc.vector.tensor_copy(RTh, pRT)
        RT255 = sm.tile([C, C], u8, tag="RT255")
        nc.vector.tensor_scalar(out=RT255, in0=pRT, scalar1=255.0,
                                scalar2=None, op0=OP.mult)

        # ---- mask byte replication via DRAM round-trip (scalar queue) ----
        # one store + one load with contiguous 4KB-per-partition reads
        rtd = dram.tile([C, C], u8)
        nc.scalar.dma_start(out=rtd[:], in_=RT255)
        rrep = wk.tile([128, C, C], u8, tag="rrep")
        rsrc = rtd[:]
        src_ap = bass.AP(tensor=rsrc.tensor, offset=rsrc.offset,
                         ap=[[0, 128], [1, C * C]])
        nc.scalar.dma_start(out=rrep, in_=src_ap)

        # ---- expert bias via RTh matmuls ----
        ps_moe = ps_moe_p.tile([128, T], f32, tag="psmoe")
        nc.tensor.matmul(ps_moe[0:C], RTh, beh, start=True, stop=False,
                         skip_group_check=True)
        nc.tensor.matmul(ps_moe[C:128], RTh, beh, start=True, stop=False,
                         skip_group_check=True)

        # lo residual of the fp8 acts (gpsimd casts, DVE subtract)
        dq = wk.tile([128, KT, 128], f32, tag="dq")
        nc.gpsimd.tensor_copy(dq, oA8[:, 0])
        lo32 = wk.tile([128, KT, 128], f32, tag="lo32")
        nc.vector.tensor_sub(lo32, oT32, dq)
        nc.gpsimd.tensor_copy(oA8[:, 1], lo32)

        # ---- expert stage: u32-AND masking + fp8 DR matmuls (hi+lo) ----
        EG = 8
        NG = C // EG
        for g in range(NG):
            e0 = g * EG
            # asc [128, EG, 2(hl), KT, 128]: one AND covers hi and lo
            asc = asc_p.tile([128, EG, 2, KT, 128], f8, tag="asc")
            out_ap = bass.AP(tensor=asc.tensor, offset=asc.offset,
                             ap=[list(asc.ap[0]), [2 * KT * 128, EG],
                                 [1, 2 * KT * 128]]).bitcast(u32)
            in0 = bass.AP(tensor=oA8.tensor, offset=oA8.offset,
                          ap=[list(oA8.ap[0]), [0, EG],
                              [1, 2 * KT * 128]]).bitcast(u32)
            rs = rrep[:, e0:e0 + EG]
            in1 = bass.AP(tensor=rs.tensor, offset=rs.offset,
                          ap=[list(rs.ap[0]), [C, EG], [0, 2 * KT * 2],
                              [1, C]]).bitcast(u32)
            nc.vector.tensor_tensor(out=out_ap, in0=in0, in1=in1,
                                    op=OP.bitwise_and)
            for i in range(EG):
                e = e0 + i
                for hl in range(2):
                    nc.tensor.matmul(ps_moe, asc[:, i, hl], we_sb[:, e],
                                     start=False,
                                     stop=(e == C - 1 and hl == 1),
                                     perf_mode=DR, skip_group_check=True)

        # ---- final stacked LN2 + residual ----
        obx = wk.tile([128, T], f32, tag="obx")
        nc.vector.tensor_tensor(out=obx, in0=OUT_LR, in1=mb_lr, op=OP.add)

        olr = wk.tile([128, T], f32, tag="olr")
        nc.vector.tensor_copy(olr, ps_moe)
        stats2 = sm.tile([128, 6], f32, tag="stats2")
        nc.vector.bn_stats(out=stats2, in_=olr)
        mv2 = sm.tile([128, 2], f32, tag="mv2")
        nc.vector.bn_aggr(out=mv2, in_=stats2)
        rstd2 = sm.tile([128, 1], f32, tag="rstd2")
        nc.scalar.activation(out=rstd2, in_=mv2[:, 1:2], func=AF.Sqrt,
                             bias=eps_t)
        nc.vector.reciprocal(rstd2, rstd2)
        nc.vector.tensor_scalar(out=olr, in0=olr, scalar1=mv2[:, 0:1],
                                scalar2=rstd2, op0=OP.subtract, op1=OP.mult)
        nc.vector.tensor_tensor(out=olr, in0=olr, in1=mg_lr, op=OP.mult)
        nc.vector.tensor_tensor(out=olr, in0=olr, in1=obx, op=OP.add)
        nc.sync.dma_start(out=olr_d.ap(), in_=olr)

    nc.compile()
    return nc


def _tile_t(w):
    t_in, n = w.shape
    return np.ascontiguousarray(w.reshape(t_in // 128, 128, n).transpose(1, 0, 2))


def _prep_in_maps(inputs):
    f = np.float32
    import ml_dtypes
    x_l, x_r = np.asarray(inputs["x_l"], f), np.asarray(inputs["x_r"], f)

    Wq = np.asarray(inputs["Wq"], f)
    Wk = np.asarray(inputs["Wk"], f)
    M = Wq.T @ Wk
    w1 = Wk.T @ np.asarray(inputs["bq"], f)

    cen = np.asarray(inputs["centers"], f)
    cenn = cen / np.maximum(np.linalg.norm(cen, axis=-1, keepdims=True), 1e-12)
    vecs = np.zeros((1, 10 * T), f)
    for i, n in enumerate(VEC_ROWS):
        src = {"bv": "bv", "bp": "bp", "agl": "ag_l", "agr": "ag_r",
               "abl": "ab_l", "abr": "ab_r", "mgl": "mg_l", "mgr": "mg_r",
               "mbl": "mb_l", "mbr": "mb_r"}[n]
        vecs[0, i * T:(i + 1) * T] = np.asarray(inputs[src], f)

    arrs = {
        "mt": _tile_t(M),
        "wvt": _tile_t(np.asarray(inputs["Wv"], f).T),
        "wpt": _tile_t(np.asarray(inputs["Wp"], f).T),
        "w1t": _tile_t(w1.reshape(T, 1)),
        "wrt": _tile_t(np.asarray(inputs["Wr"], f).T),
        "brp": np.asarray(inputs["br"], f).reshape(EXP, 1),
        "cent": np.ascontiguousarray(cenn.T),
        "ident": np.eye(128, dtype=f),
    }
    We = np.asarray(inputs["We"], f) * WE_SCALE
    WeTh = np.ascontiguousarray(
        We.transpose(0, 2, 1).reshape(C, KT, 128, T).transpose(2, 0, 1, 3)
    ).astype(ml_dtypes.float8_e4m3)
    beh = (np.asarray(inputs["be"], f) * WE_SCALE).astype(np.float16)

    def pack(spec, ncols, extra):
        blob = np.zeros((128, ncols), f)
        for name, parts, shape in spec:
            off, _, _ = BLOB_OFF[name]
            cols = int(np.prod(shape[1:]))
            a = extra[name] if name in extra else arrs[name]
            blob[0:parts, off:off + cols] = np.asarray(a, f).reshape(parts, cols)
        return blob

    blobB = pack(BLOB_B_SPEC, NB_COLS, {})
    in_maps = []
    for b in range(N_CORES):
        xtl = _tile_t(np.ascontiguousarray(x_l[b].T))
        xtr = _tile_t(np.ascontiguousarray(x_r[b].T))
        xlr = np.concatenate([x_l[b], x_r[b]], axis=0)
        blobA0 = pack(BLOB_A0_SPEC, NA0_COLS, {"xtl": xtl})
        blobA = pack(BLOB_A_SPEC, NA_COLS, {"xtr": xtr, "xlr": xlr})
        in_maps.append({"blobA0": blobA0, "blobA": blobA, "blobB": blobB,
                        "vecsd": vecs, "weh": WeTh, "beh": beh})
    return in_maps


def kernel(**inputs) -> np.ndarray:
    from concourse.bass_utils import run_bass_kernel_spmd

    if "nc" not in _CACHE:
        _CACHE["nc"] = _build()
    nc = _CACHE["nc"]
    in_maps = _prep_in_maps(inputs)
    res = run_bass_kernel_spmd(nc, in_maps, list(range(N_CORES)))
    _CACHE["exec_time_ns"] = res.exec_time_ns
    olr = np.stack([res.results[b]["olr"] for b in range(N_CORES)])
    return np.stack([olr[:, 0:C, :], olr[:, C:128, :]]).astype(np.float32)


# revision 33
# speedup vs baseline: 1.2019x; 1.0176x over previous
"""Trainium2 Bass kernel for nn_BiDGNBlock (moe_routing).

Data-parallel over batch across 8 NeuronCores (no collectives). Each core
computes one batch element end-to-end.

Structure (v2):
  - Expert table We streamed as fp8e4m3 (x64 prescaled; final LN is
    scale-invariant) -> 4.2MB. Expert matmuls in DoubleRow fp8 perf mode.
    Activations split hi+lo fp8e4m3 (two DR passes) so only the weight
    quantization error remains (~3.6% RMS -> ~1.5e-2 final rel).
  - Attention computes transposed outputs directly (out.T = v.T-style
    matmuls against scaled expE), with the softmax denominator and the
    bias-fold factors applied as per-row scalings folded into the
    projection stage. Both sides stacked as 128 rows everywhere.
  - Router: top-2 membership is invariant to the xp row norms, so the
    whole normalization chain is skipped; masks come from
    sim >= second_max per channel (is_ge), transposed once on the PE.
  - Mask bytes (0x00/0xFF) replicated across partitions via a small DRAM
    round-trip on the scalar engine's DMA queue; applied to fp8 acts with
    uint32 bitwise-AND ops on the DVE.
  - Replicated LN vectors loaded with partition-broadcast DMA reads on the
    scalar queue (no PE/DVE cost, no host-side replication bytes).
Routing stays exact-fp32 end-to-end.
"""

import sys
import numpy as np

sys.path.insert(0, "/opt/trn_rl_repo")

N_CORES = 8
B, C, T = 8, 64, 256
EXP = 32
KT = T // 128
WE_SCALE = 64.0

_CACHE: dict = {}

BLOB_A0_SPEC = [
    ("xtl", 128, (128, KT, C)), ("mt", 128, (128, KT, T)),
]
BLOB_A_SPEC = [
    ("xtr", 128, (128, KT, C)), ("wvt", 128, (128, KT, T)),
    ("xlr", 128, (128, T)), ("ident", 128, (128, 128)),
    ("w1t", 128, (128, KT, 1)),
]
BLOB_B_SPEC = [
    ("wpt", 128, (128, KT, T)), ("wrt", 128, (128, 2 * KT, EXP)),
    ("brp", 32, (32, 1)), ("cent", 32, (32, C)),
]
VEC_ROWS = ["bv", "bp", "agl", "agr", "abl", "abr", "mgl", "mgr",
            "mbl", "mbr"]


def _blob_layout():
    off = {}
    na0 = 0
    for name, parts, shape in BLOB_A0_SPEC:
        cols = int(np.prod(shape[1:]))
        off[name] = (na0, parts, shape)
        na0 += cols
    na = 0
    for name, parts, shape in BLOB_A_SPEC:
        cols = int(np.prod(shape[1:]))
        off[name] = (na, parts, shape)
        na += cols
    nb = 0
    for name, parts, shape in BLOB_B_SPEC:
        cols = int(np.prod(shape[1:]))
        off[name] = (nb, parts, shape)
        nb += cols
    return off, na0, na, nb


BLOB_OFF, NA0_COLS, NA_COLS, NB_COLS = _blob_layout()


def _build():
    import concourse.bass as bass
    import concourse.mybir as mybir
    import concourse.tile as tile
    from concourse import bacc
    from contextlib import ExitStack

    dt = mybir.dt
    f32, f16, f8, u8 = dt.float32, dt.float16, dt.float8e4, dt.uint8
    u32 = dt.uint32
    AF = mybir.ActivationFunctionType
    OP = mybir.AluOpType
    DR = mybir.MatmulPerfMode.DoubleRow

    nc = bacc.Bacc("TRN2", target_bir_lowering=False, debug=False,
                   num_devices=N_CORES)

    def inp(name, shape, d=f32):
        return nc.dram_tensor(name, list(shape), d, kind="ExternalInput")

    blobA0_d = inp("blobA0", (128, NA0_COLS))
    blobA_d = inp("blobA", (128, NA_COLS))
    blobB_d = inp("blobB", (128, NB_COLS))
    vecs_d = inp("vecsd", (1, 10 * T))
    weh_d = inp("weh", (128, C, KT, T), f8)
    beh_d = inp("beh", (C, T), f16)

    olr_d = nc.dram_tensor("olr", [128, T], f32, kind="ExternalOutput")

    with tile.TileContext(nc) as tc, ExitStack() as ctx:
        cst = ctx.enter_context(tc.tile_pool(name="cst", bufs=1))
        wk = ctx.enter_context(tc.tile_pool(name="wk", bufs=2))
        sm = ctx.enter_context(tc.tile_pool(name="sm", bufs=2))
        asc_p = ctx.enter_context(tc.tile_pool(name="asc", bufs=3))
        ps = ctx.enter_context(tc.tile_pool(name="ps", bufs=3, space="PSUM"))
        ps_moe_p = ctx.enter_context(tc.tile_pool(name="psmoe", bufs=1,
                                                  space="PSUM"))
        dram = ctx.enter_context(tc.tile_pool(name="dram", bufs=1,
                                              space="DRAM"))

        # ---- input DMAs: attention-critical blobA0 first on sync queue ----
        blobA0 = cst.tile([128, NA0_COLS], f32, tag="blobA0")
        nc.sync.dma_start(out=blobA0, in_=blobA0_d.ap())
        blobA = cst.tile([128, NA_COLS], f32, tag="blobA")
        nc.sync.dma_start(out=blobA, in_=blobA_d.ap())
        blobB = cst.tile([128, NB_COLS], f32, tag="blobB")
        nc.sync.dma_start(out=blobB, in_=blobB_d.ap())
        we_sb = cst.tile([128, C, KT, T], f8, tag="weh")
        wea = weh_d.ap()
        for ch in range(8):
            nc.sync.dma_start(out=we_sb[:, ch * 8:(ch + 1) * 8],
                              in_=wea[:, ch * 8:(ch + 1) * 8])
        # small loads on the (otherwise idle) scalar queue
        vecs_sb = cst.tile([1, 10 * T], f32, tag="vecs")
        nc.scalar.dma_start(out=vecs_sb, in_=vecs_d.ap())
        beh = cst.tile([C, T], f16, tag="beh")
        nc.scalar.dma_start(out=beh, in_=beh_d.ap())

        # replicated LN vectors via gpsimd partition_broadcast (no DMA, no PE).
        # NOTE: on HW the broadcast only works with dst base partition 0, so
        # stacked l|r tiles broadcast the l-vector to all 128 rows and then
        # overwrite rows 64-127 with a Pool copy from an r-scratch tile.
        vscr = cst.tile([128, T], f32, tag="vscr")

        def vec_rep2(i_l, i_r, tag):
            t_sb = cst.tile([128, T], f32, tag=tag)
            nc.gpsimd.partition_broadcast(
                t_sb, vecs_sb[0:1, i_l * T:(i_l + 1) * T])
            if i_r != i_l:
                nc.gpsimd.partition_broadcast(
                    vscr, vecs_sb[0:1, i_r * T:(i_r + 1) * T])
                nc.gpsimd.tensor_copy(t_sb[64:128], vscr[64:128])
            return t_sb

        # order by consumer time: bv gates v (early), bp/g/b gate LN1,
        # mg/mb only gate LN2 (late)
        bv_t = cst.tile([64, T], f32, tag="bv")
        nc.gpsimd.partition_broadcast(bv_t, vecs_sb[0:1, 0:T])
        bp_t = vec_rep2(1, 1, "bp")
        g_lr = vec_rep2(2, 3, "glr")
        b_lr = vec_rep2(4, 5, "blr")
        mg_lr = vec_rep2(6, 7, "mglr")
        mb_lr = vec_rep2(8, 9, "mblr")

        def bview(blob, name):
            off, parts, shape = BLOB_OFF[name]
            cols = 1
            for s in shape[1:]:
                cols *= s
            v = blob[0:parts, off:off + cols]
            if len(shape) == 3:
                v = v.rearrange("p (a b) -> p a b", a=shape[1])
            return v

        xtl = bview(blobA0, "xtl")
        mt = bview(blobA0, "mt")
        xtr = bview(blobA, "xtr")
        wvt = bview(blobA, "wvt")
        xlr = bview(blobA, "xlr")
        ident = bview(blobA, "ident")
        w1t = bview(blobA, "w1t")
        wpt = bview(blobB, "wpt")
        wrt = bview(blobB, "wrt")
        brp = bview(blobB, "brp")
        cent = bview(blobB, "cent")

        eps_t = cst.tile([128, 1], f32, tag="eps")
        nc.vector.memset(eps_t, 1e-5)
        onescol = cst.tile([64, 1], f32, tag="onescol")
        nc.vector.memset(onescol, 1.0)

        # PE warm-up + ACT table preloads during the DMA window
        warm_p = ctx.enter_context(tc.tile_pool(name="warm", bufs=1,
                                                space="PSUM"))
        wsrc = cst.tile([128, 512], f16, tag="wsrc")
        nc.vector.memset(wsrc, 0.5)
        pw = warm_p.tile([128, 512], f32, tag="warm")
        for wi in range(6):
            nc.tensor.matmul(pw, wsrc[:, 0:128], wsrc,
                             start=True, stop=True, skip_group_check=True)
        # preload only the Exp table now; Sqrt is prefetched later via a
        # dummy activation so the Exp->Sqrt switch happens off-path exactly
        # once (table loads cost 1.28us on the ACT engine).
        wact = cst.tile([1, 32], f32, tag="wact")
        nc.vector.memset(wact, 1.0)
        nc.scalar.activation(out=wact, in_=wact, func=AF.Exp)

        # ---- attention ----
        # A.T = (x_l @ M).T  [u, kt, cq]
        AT = wk.tile([128, KT, C], f32, tag="AT")
        for uo in range(KT):
            p = ps.tile([128, C], f32, tag="ps")
            for kt in range(KT):
                nc.tensor.matmul(p, mt[:, kt, uo * 128:(uo + 1) * 128],
                                 xtl[:, kt], start=(kt == 0),
                                 stop=(kt == KT - 1))
            nc.vector.tensor_copy(AT[:, uo], p)
        # energy E = A @ x_r.T
        pe_ = ps.tile([C, C], f32, tag="ps")
        for kt in range(KT):
            nc.tensor.matmul(pe_, AT[:, kt], xtr[:, kt],
                             start=(kt == 0), stop=(kt == KT - 1))
        # expE = exp(E/16)  (|E|/16 small enough to skip max-subtract)
        expE = wk.tile([C, C], f32, tag="expE")
        nc.scalar.activation(out=expE, in_=pe_, func=AF.Exp, scale=1.0 / 16.0)
        # rv.T = x_r @ (Wk.T bq); grv = exp(rv/16)  (bias fold, g-column)
        prv = ps.tile([C, 1], f32, tag="ps")
        for kt in range(KT):
            nc.tensor.matmul(prv, xtr[:, kt], w1t[:, kt],
                             start=(kt == 0), stop=(kt == KT - 1))
        grv = sm.tile([C, 1], f32, tag="grv")
        nc.scalar.activation(out=grv, in_=prv, func=AF.Exp, scale=1.0 / 16.0)
        # prefetch the Sqrt table while the DVE runs the LN1 stats
        nc.scalar.activation(out=wact, in_=wact, func=AF.Sqrt)
        # v = (x_l - x_r) @ Wv.T + bv   [ck, u]
        xdt = wk.tile([128, KT, C], f32, tag="xdt")
        nc.vector.tensor_sub(xdt, xtl, xtr)
        pv = ps.tile([C, T], f32, tag="ps")
        for kt in range(KT):
            nc.tensor.matmul(pv, xdt[:, kt], wvt[:, kt],
                             start=(kt == 0), stop=(kt == KT - 1))
        v_sb = wk.tile([C, T], f32, tag="v")
        nc.vector.tensor_tensor(out=v_sb, in0=pv, in1=bv_t, op=OP.add)

        # expEg_T[ck, cq] = expE[cq, ck] * g[ck]  (transpose + g-fold)
        pet = ps.tile([C, C], f32, tag="ps")
        nc.tensor.transpose(pet, expE, ident[0:C, 0:C])
        expEgT = wk.tile([C, C], f32, tag="expEgT")
        nc.vector.tensor_scalar(out=expEgT, in0=pet, scalar1=grv,
                                scalar2=None, op0=OP.mult)
        # S[cq] = col sums of expEg_T;  recipL = 1/S
        pS = ps.tile([C, 1], f32, tag="ps")
        nc.tensor.matmul(pS, expEgT, onescol, start=True, stop=True)
        recipL = sm.tile([C, 1], f32, tag="recipL")
        nc.vector.reciprocal(recipL, pS)
        # expEn[l, q] = expE[l, q] / S[l]
        expEn = wk.tile([C, C], f32, tag="expEn")
        nc.vector.tensor_scalar(out=expEn, in0=expE, scalar1=recipL,
                                scalar2=None, op0=OP.mult)
        # scaleLR = [recipL | grv]
        scaleLR = sm.tile([128, 1], f32, tag="scaleLR")
        nc.vector.tensor_copy(scaleLR[0:64], recipL)
        nc.vector.tensor_copy(scaleLR[64:128], grv)

        # transposed attention outputs: aoT [u, kt, rows]  (rows = l | r)
        aoT = wk.tile([128, KT, 128], f32, tag="aoT")
        for ut in range(KT):
            pl = ps.tile([128, C], f32, tag="ps")
            nc.tensor.matmul(pl, v_sb[:, ut * 128:(ut + 1) * 128], expEgT,
                             start=True, stop=True)
            nc.vector.tensor_copy(aoT[:, ut, 0:64], pl)
            pr = ps.tile([128, C], f32, tag="ps")
            nc.tensor.matmul(pr, v_sb[:, ut * 128:(ut + 1) * 128], expEn,
                             start=True, stop=True)
            nc.vector.tensor_copy(aoT[:, ut, 64:128], pr)

        # beta + residual precombine (off critical path)
        bx_lr = cst.tile([128, T], f32, tag="bxlr")
        nc.vector.tensor_add(bx_lr, b_lr, xlr)

        # ---- proj + stacked LN1 + residual -> OUT_LR [128 rows, T] ----
        pp = ps.tile([128, T], f32, tag="ps")
        for kt in range(KT):
            nc.tensor.matmul(pp, aoT[:, kt], wpt[:, kt],
                             start=(kt == 0), stop=(kt == KT - 1))
        OUT_LR = wk.tile([128, T], f32, tag="OUTLR")
        # (pp * scaleLR) + bp   (row scale folds softmax denom / bias terms)
        nc.vector.scalar_tensor_tensor(out=OUT_LR, in0=pp, scalar=scaleLR,
                                       in1=bp_t, op0=OP.mult, op1=OP.add)
        stats = sm.tile([128, 6], f32, tag="stats1")
        nc.vector.bn_stats(out=stats, in_=OUT_LR)
        mv = sm.tile([128, 2], f32, tag="mv1")
        nc.vector.bn_aggr(out=mv, in_=stats)
        rstd = sm.tile([128, 1], f32, tag="rstd1")
        nc.scalar.activation(out=rstd, in_=mv[:, 1:2], func=AF.Sqrt,
                             bias=eps_t)
        nc.vector.reciprocal(rstd, rstd)
        nc.vector.tensor_scalar(out=OUT_LR, in0=OUT_LR, scalar1=mv[:, 0:1],
                                scalar2=rstd, op0=OP.subtract, op1=OP.mult)
        nc.vector.tensor_tensor(out=OUT_LR, in0=OUT_LR, in1=g_lr, op=OP.mult)
        nc.vector.tensor_tensor(out=OUT_LR, in0=OUT_LR, in1=bx_lr, op=OP.add)

        # ---- transposes for router/experts: oT32 [u, kt, rows] ----
        oT32 = wk.tile([128, KT, 128], f32, tag="oT32")
        for kt in range(KT):
            pt = ps.tile([128, 128], f32, tag="ps")
            nc.tensor.transpose(pt, OUT_LR[:, kt * 128:(kt + 1) * 128], ident)
            nc.vector.tensor_copy(oT32[:, kt], pt)
        # fp8 hi copy of activations (gpsimd: no ACT tables, Pool is idle)
        # oA8 holds [hi | lo] halves: [128, 2(hl), KT, 128]
        oA8 = wk.tile([128, 2, KT, 128], f8, tag="oA8")
        nc.gpsimd.tensor_copy(oA8[:, 0], oT32)

        # ---- router -> sims -> top-2 membership masks ----
        pxp = ps.tile([EXP, C], f32, tag="ps")
        j = 0
        for side in range(2):
            for kt in range(KT):
                nc.tensor.matmul(pxp, wrt[:, side * KT + kt],
                                 oT32[:, kt, side * 64:(side + 1) * 64],
                                 start=(j == 0), stop=(j == 3))
                j += 1
        xpT = wk.tile([EXP, C], f32, tag="xpT")
        nc.vector.tensor_scalar(out=xpT, in0=pxp, scalar1=brp, scalar2=None,
                                op0=OP.add)
        # sims (unnormalized; top-2 membership is row-norm invariant)
        psim = ps.tile([C, C], f32, tag="ps")
        nc.tensor.matmul(psim, xpT, cent, start=True, stop=True)
        mx8 = sm.tile([C, 8], f32, tag="mx8")
        nc.vector.max(out=mx8, in_=psim)
        # R[c, e] = sim[c, e] >= second_max[c]
        Rcm = sm.tile([C, C], f32, tag="Rcm")
        nc.vector.tensor_scalar(out=Rcm, in0=psim, scalar1=mx8[:, 1:2],
                                scalar2=None, op0=OP.is_ge)
        pRT = ps.tile([C, C], f32, tag="ps")
        nc.tensor.transpose(pRT, Rcm, ident[0:C, 0:C])
        RTh = sm.tile([C, C], f16, tag="RTh")
        nc.vector.tensor_copy(RTh, pRT)
        RT255 = sm.tile([C, C], u8, tag="RT255")
        nc.vector.tensor_scalar(out=RT255, in0=pRT, scalar1=255.0,
                                scalar2=None, op0=OP.mult)

        # ---- mask byte replication: DRAM flatten (4KB) + Pool broadcast ----
        rtd = dram.tile([C, C], u8)
        nc.scalar.dma_start(out=rtd[:], in_=RT255)
        rflat = wk.tile([1, C * C], u8, tag="rflat")
        rsrc = rtd[:]
        src_ap = bass.AP(tensor=rsrc.tensor, offset=rsrc.offset,
                         ap=[[0, 1], [1, C * C]])
        nc.scalar.dma_start(out=rflat, in_=src_ap)
        rrep = wk.tile([128, C, C], u8, tag="rrep")
        rrep_b = bass.AP(tensor=rrep.tensor, offset=rrep.offset,
                         ap=[list(rrep.ap[0]), [1, C * C]]).bitcast(u32)
        rflat_b = rflat[0:1, 0:C * C].bitcast(u32)
        nc.gpsimd.partition_broadcast(rrep_b, rflat_b)

        # ---- expert bias via RTh matmuls ----
        ps_moe = ps_moe_p.tile([128, T], f32, tag="psmoe")
        nc.tensor.matmul(ps_moe[0:C], RTh, beh, start=True, stop=False,
                         skip_group_check=True)
        nc.tensor.matmul(ps_moe[C:128], RTh, beh, start=True, stop=False,
                         skip_group_check=True)

        # lo residual of the fp8 acts (gpsimd casts, DVE subtract)
        dq = wk.tile([128, KT, 128], f32, tag="dq")
        nc.gpsimd.tensor_copy(dq, oA8[:, 0])
        lo32 = wk.tile([128, KT, 128], f32, tag="lo32")
        nc.vector.tensor_sub(lo32, oT32, dq)
        nc.gpsimd.tensor_copy(oA8[:, 1], lo32)

        # ---- expert stage: u32-AND masking + fp8 DR matmuls (hi+lo) ----
        EG = 8
        NG = C // EG
        for g in range(NG):
            e0 = g * EG
            # asc [128, EG, 2(hl), KT, 128]: one AND covers hi and lo
            asc = asc_p.tile([128, EG, 2, KT, 128], f8, tag="asc")
            out_ap = bass.AP(tensor=asc.tensor, offset=asc.offset,
                             ap=[list(asc.ap[0]), [2 * KT * 128, EG],
                                 [1, 2 * KT * 128]]).bitcast(u32)
            in0 = bass.AP(tensor=oA8.tensor, offset=oA8.offset,
                          ap=[list(oA8.ap[0]), [0, EG],
                              [1, 2 * KT * 128]]).bitcast(u32)
            rs = rrep[:, e0:e0 + EG]
            in1 = bass.AP(tensor=rs.tensor, offset=rs.offset,
                          ap=[list(rs.ap[0]), [C, EG], [0, 2 * KT * 2],
                              [1, C]]).bitcast(u32)
            nc.vector.tensor_tensor(out=out_ap, in0=in0, in1=in1,
                                    op=OP.bitwise_and)
            for i in range(EG):
                e = e0 + i
                for hl in range(2):
                    nc.tensor.matmul(ps_moe, asc[:, i, hl], we_sb[:, e],
                                     start=False,
                                     stop=(e == C - 1 and hl == 1),
                                     perf_mode=DR, skip_group_check=True)

        # ---- final stacked LN2 + residual ----
        obx = wk.tile([128, T], f32, tag="obx")
        nc.vector.tensor_tensor(out=obx, in0=OUT_LR, in1=mb_lr, op=OP.add)

        olr = wk.tile([128, T], f32, tag="olr")
        nc.vector.tensor_copy(olr, ps_moe)
        stats2 = sm.tile([128, 6], f32, tag="stats2")
        nc.vector.bn_stats(out=stats2, in_=olr)
        mv2 = sm.tile([128, 2], f32, tag="mv2")
        nc.vector.bn_aggr(out=mv2, in_=stats2)
        rstd2 = sm.tile([128, 1], f32, tag="rstd2")
        nc.scalar.activation(out=rstd2, in_=mv2[:, 1:2], func=AF.Sqrt,
                             bias=eps_t)
        nc.vector.reciprocal(rstd2, rstd2)
        nc.vector.tensor_scalar(out=olr, in0=olr, scalar1=mv2[:, 0:1],
                                scalar2=rstd2, op0=OP.subtract, op1=OP.mult)
        nc.vector.tensor_tensor(out=olr, in0=olr, in1=mg_lr, op=OP.mult)
        nc.vector.tensor_tensor(out=olr, in0=olr, in1=obx, op=OP.add)
        nc.sync.dma_start(out=olr_d.ap(), in_=olr)

    nc.compile()
    return nc


def _tile_t(w):
    t_in, n = w.shape
    return np.ascontiguousarray(w.reshape(t_in // 128, 128, n).transpose(1, 0, 2))


def _prep_in_maps(inputs):
    f = np.float32
    import ml_dtypes
    x_l, x_r = np.asarray(inputs["x_l"], f), np.asarray(inputs["x_r"], f)

    Wq = np.asarray(inputs["Wq"], f)
    Wk = np.asarray(inputs["Wk"], f)
    M = Wq.T @ Wk
    w1 = Wk.T @ np.asarray(inputs["bq"], f)

    cen = np.asarray(inputs["centers"], f)
    cenn = cen / np.maximum(np.linalg.norm(cen, axis=-1, keepdims=True), 1e-12)
    vecs = np.zeros((1, 10 * T), f)
    for i, n in enumerate(VEC_ROWS):
        src = {"bv": "bv", "bp": "bp", "agl": "ag_l", "agr": "ag_r",
               "abl": "ab_l", "abr": "ab_r", "mgl": "mg_l", "mgr": "mg_r",
               "mbl": "mb_l", "mbr": "mb_r"}[n]
        vecs[0, i * T:(i + 1) * T] = np.asarray(inputs[src], f)

    arrs = {
        "mt": _tile_t(M),
        "wvt": _tile_t(np.asarray(inputs["Wv"], f).T),
        "wpt": _tile_t(np.asarray(inputs["Wp"], f).T),
        "w1t": _tile_t(w1.reshape(T, 1)),
        "wrt": _tile_t(np.asarray(inputs["Wr"], f).T),
        "brp": np.asarray(inputs["br"], f).reshape(EXP, 1),
        "cent": np.ascontiguousarray(cenn.T),
        "ident": np.eye(128, dtype=f),
    }
    We = np.asarray(inputs["We"], f) * WE_SCALE
    WeTh = np.ascontiguousarray(
        We.transpose(0, 2, 1).reshape(C, KT, 128, T).transpose(2, 0, 1, 3)
    ).astype(ml_dtypes.float8_e4m3)
    beh = (np.asarray(inputs["be"], f) * WE_SCALE).astype(np.float16)

    def pack(spec, ncols, extra):
        blob = np.zeros((128, ncols), f)
        for name, parts, shape in spec:
            off, _, _ = BLOB_OFF[name]
            cols = int(np.prod(shape[1:]))
            a = extra[name] if name in extra else arrs[name]
            blob[0:parts, off:off + cols] = np.asarray(a, f).reshape(parts, cols)
        return blob

    blobB = pack(BLOB_B_SPEC, NB_COLS, {})
    in_maps = []
    for b in range(N_CORES):
        xtl = _tile_t(np.ascontiguousarray(x_l[b].T))
        xtr = _tile_t(np.ascontiguousarray(x_r[b].T))
        xlr = np.concatenate([x_l[b], x_r[b]], axis=0)
        blobA0 = pack(BLOB_A0_SPEC, NA0_COLS, {"xtl": xtl})
        blobA = pack(BLOB_A_SPEC, NA_COLS, {"xtr": xtr, "xlr": xlr})
        in_maps.append({"blobA0": blobA0, "blobA": blobA, "blobB": blobB,
                        "vecsd": vecs, "weh": WeTh, "beh": beh})
    return in_maps


def kernel(**inputs) -> np.ndarray:
    from concourse.bass_utils import run_bass_kernel_spmd

    if "nc" not in _CACHE:
        _CACHE["nc"] = _build()
    nc = _CACHE["nc"]
    in_maps = _prep_in_maps(inputs)
    res = run_bass_kernel_spmd(nc, in_maps, list(range(N_CORES)))
    _CACHE["exec_time_ns"] = res.exec_time_ns
    olr = np.stack([res.results[b]["olr"] for b in range(N_CORES)])
    return np.stack([olr[:, 0:C, :], olr[:, C:128, :]]).astype(np.float32)
